# revision 19
# baseline (speedup 1.0000x reference)
"""Self-contained TRN2 kernel for nn_FLASH_ShareA_FFConvM_FlashAttn.

kernel(**inputs) takes the full (unsharded) inputs from setup_inputs() and
returns the full (B, N, D) float32 output. Internally: data-parallel over the
batch — one batch sample per NeuronCore, 8 cores, no collectives.

v2: depthwise convs via fp8 DoubleRow diag matmuls on the PE; z_o kept
SBUF-resident (no DRAM roundtrip); batched Sqrt in the gating phase to avoid
activation-table thrash; shift copies on the scalar engine.
"""
import sys

if "/opt/trn_rl_repo" not in sys.path:
    sys.path.insert(0, "/opt/trn_rl_repo")

import numpy as np
import ml_dtypes
import concourse.bass as bass
import concourse.bacc as bacc
import concourse.mybir as mybir
import concourse.tile as tile
from concourse import bass_utils

F32 = mybir.dt.float32
BF16 = mybir.dt.bfloat16
FP8 = mybir.dt.float8e4
AF = mybir.ActivationFunctionType
OP = mybir.AluOpType
PM = mybir.MatmulPerfMode

N, D, H, QK, G = 4096, 512, 2048, 128, 256
NG = N // G
NT = N // 128
KTAPS = 17
PAD = 8
E2 = 2 * D
EPS = 1e-5
W8 = 2 * PAD + N + 16  # fp8 double-plane buffer width (4128, mult of 16)

# channel tiles of the depthwise convs on the PE (fp8 DoubleRow diag matmuls);
# the rest run on the vector engine. DVE tiles are scheduled early so no
# phase ends on a long vector-engine tail.
CONV_PE_HID = frozenset(range(16)) - {5, 9, 13}
CONV_PE_QK = True
CONV_PE_O = frozenset({0, 2, 3})
# DoubleRow tap pairs (k, k+2) matching a +2-column plane-1 shift
PAIR_KS = [0, 1, 4, 5, 8, 9, 12, 13]


def _conv_dve(nc, scratch, hpad, hpad1, dw_sb, dwi, acc):
    """acc = h + conv(h) via tensor_scalar products (4x) + tensor_tensor adds (2x)."""
    for k in range(KTAPS):
        s = k - PAD
        if s % 2 == 0:
            src, off = hpad, PAD + s
        else:
            src, off = hpad1, PAD - 1 + s
        if k == 0:
            nc.vector.scalar_tensor_tensor(
                out=acc[:, :], in0=src[:, off:off + N], scalar=dw_sb[:, dwi, 0:1],
                in1=hpad[:, PAD:PAD + N], op0=OP.mult, op1=OP.add)
        else:
            nc.vector.tensor_scalar(out=scratch[:, :], in0=src[:, off:off + N],
                                    scalar1=dw_sb[:, dwi, k:k + 1], scalar2=None,
                                    op0=OP.mult)
            nc.vector.tensor_add(acc[:, :], acc[:, :], scratch[:, :])


def _build_hp8(nc, pool, hpad):
    """fp8 double-plane buffer: plane0 = fp8(hpad), plane1 = plane0 shifted +2."""
    hp8 = pool.tile([128, 2, W8], FP8, tag="hp8")
    nc.scalar.activation(hp8[:, 0, 0:2 * PAD + N], hpad[:, :], AF.Copy)
    nc.vector.memset(hp8[:, 0, 2 * PAD + N:W8], 0.0)
    nc.sync.dma_start(hp8[:, 1, 0:W8 - 2], hp8[:, 0, 2:W8])
    nc.vector.memset(hp8[:, 1, W8 - 2:W8], 0.0)
    return hp8


def _build_dg(nc, pool, diag_mask, dw_sb, dwi):
    """fp8 diag weight pairs [128, 9, 2, 128] for DoubleRow conv."""
    dg = pool.tile([128, 9, 2, 128], FP8, tag="dg")
    for j, k in enumerate(PAIR_KS):
        nc.vector.tensor_scalar(out=dg[:, j, 0, :], in0=diag_mask[:, :],
                                scalar1=dw_sb[:, dwi, k:k + 1], scalar2=None, op0=OP.mult)
        nc.vector.tensor_scalar(out=dg[:, j, 1, :], in0=diag_mask[:, :],
                                scalar1=dw_sb[:, dwi, k + 2:k + 3], scalar2=None, op0=OP.mult)
    nc.vector.tensor_scalar(out=dg[:, 8, 0, :], in0=diag_mask[:, :],
                            scalar1=dw_sb[:, dwi, 16:17], scalar2=None, op0=OP.mult)
    nc.vector.memset(dg[:, 8, 1, :], 0.0)
    return dg


def _conv_dr(nc, psum_pool, hpad, hp8, dg, diag_mask, acc, evac):
    """acc = h + conv(h): bf16 identity + 9 fp8 DoubleRow diag pair-matmuls."""
    for half in range(4):
        ps0 = psum_pool.tile([128, 512], F32, tag="convdr")
        ps1 = psum_pool.tile([128, 512], F32, tag="convdr")
        ps = [ps0, ps1]
        bases = [PAD + (2 * half + i) * 512 for i in range(2)]
        for i in range(2):
            nc.tensor.matmul(ps[i][:, :], diag_mask[:, :], hpad[:, bases[i]:bases[i] + 512],
                             start=True, stop=False, skip_group_check=True)
        for j, k in enumerate(PAIR_KS + [16]):
            s = k - PAD
            last = j == 8
            for i in range(2):
                nc.tensor.matmul(ps[i][:, :], dg[:, j, :, :],
                                 hp8[:, :, bases[i] + s:bases[i] + s + 512],
                                 start=False, stop=last, perf_mode=PM.DoubleRow,
                                 skip_group_check=True)
        for i in range(2):
            c = 2 * half + i
            evac.activation(acc[:, c * 512:(c + 1) * 512], ps[i][:, :], AF.Copy)


def _emit(nc, tc, x, wh, wqk, wo, bh, bqk, bo, dwh, dwqk, dwo, gb, out, spill):
    consts = tc.alloc_tile_pool(name="consts", bufs=1)
    wqk_sb = consts.tile([128, 4, QK], BF16)
    nc.sync.dma_start(wqk_sb[:, :, :], wqk.ap())
    wo_sb = consts.tile([128, 8, D], BF16)
    nc.sync.dma_start(wo_sb[:, :, :], wo.ap())
    bh_sb = consts.tile([128, 16], F32)
    nc.sync.dma_start(bh_sb[:, :], bh.ap())
    bqk_sb = consts.tile([128, 1], F32)
    nc.sync.dma_start(bqk_sb[:, :], bqk.ap())
    bo_sb = consts.tile([128, 4], F32)
    nc.sync.dma_start(bo_sb[:, :], bo.ap())
    dwh_sb = consts.tile([128, 16, KTAPS], F32)
    nc.sync.dma_start(dwh_sb[:, :, :], dwh.ap())
    dwqk_sb = consts.tile([128, 1, KTAPS], F32)
    nc.sync.dma_start(dwqk_sb[:, :, :], dwqk.ap())
    dwo_sb = consts.tile([128, 4, KTAPS], F32)
    nc.sync.dma_start(dwo_sb[:, :, :], dwo.ap())
    gb_sb = consts.tile([128, 8], F32)
    nc.sync.dma_start(gb_sb[:, :], gb.ap())
    eps_sb = consts.tile([128, 1], F32)
    nc.vector.memset(eps_sb[:, :], EPS)

    iota_row = consts.tile([128, 128], F32)
    nc.gpsimd.iota(iota_row[:, :], pattern=[[1, 128]], base=0, channel_multiplier=0,
                   allow_small_or_imprecise_dtypes=True)
    iota_p = consts.tile([128, 1], F32)
    nc.gpsimd.iota(iota_p[:, :], pattern=[[0, 1]], base=0, channel_multiplier=1,
                   allow_small_or_imprecise_dtypes=True)
    diag_mask = consts.tile([128, 128], BF16)
    nc.vector.tensor_scalar(out=diag_mask[:, :], in0=iota_row[:, :],
                            scalar1=iota_p[:, :], scalar2=None, op0=OP.is_equal)

    p03 = tc.alloc_tile_pool(name="p03", bufs=1)     # P0-P3: zT
    zT = p03.tile([128, 4, N], BF16)
    qs = tc.alloc_tile_pool(name="qside", bufs=1, side="right")    # P1-P4
    attnT = qs.tile([128, NG, 2, G], BF16)
    lq_sb = qs.tile([128, N], BF16)
    lk_str = qs.tile([128, NT, 128], BF16)
    linkv_sb = qs.tile([128, E2], BF16)
    linku_sb = qs.tile([128, E2], BF16)

    # P0 + P1 head: token-shifted LayerNorm, with the qk GEMM interleaved
    # per 512-token chunk (keeps the PE warm during the load/normalize phase).
    with tc.tile_pool(name="p0", bufs=3) as p0, \
         tc.tile_pool(name="p0s", bufs=8) as p0s, \
         tc.tile_pool(name="p1", bufs=1) as p1, \
         tc.tile_pool(name="p1p", bufs=2, space="PSUM") as p1p:
        qkp = p1.tile([128, 2 * PAD + N], BF16, tag="qkpad")
        nc.vector.memset(qkp[:, 0:PAD], 0.0)
        nc.vector.memset(qkp[:, PAD + N:], 0.0)
        for ch in range(8):
            t0 = ch * 512
            xt4 = p0.tile([128, 4, D], F32, tag="xt4")
            if ch == 0:
                nc.vector.memset(xt4[0:1, 0, 0:D // 2], 0.0)
                nc.sync.dma_start(xt4[1:128, 0, 0:D // 2], x[0:127, 0:D // 2])
                nc.sync.dma_start(
                    xt4[:, 1:4, 0:D // 2],
                    x[127:511, 0:D // 2].rearrange("(q p) d -> p q d", p=128))
            else:
                nc.sync.dma_start(
                    xt4[:, :, 0:D // 2],
                    x[t0 - 1:t0 + 511, 0:D // 2].rearrange("(q p) d -> p q d", p=128))
            nc.sync.dma_start(
                xt4[:, :, D // 2:D],
                x[t0:t0 + 512, D // 2:D].rearrange("(q p) d -> p q d", p=128))
            mv4 = p0s.tile([128, 4, 2], F32, tag="mv4p0")
            for q in range(4):
                st6 = p0s.tile([128, 6], F32, tag="st6")
                nc.vector.bn_stats(st6[:, :], xt4[:, q, :])
                nc.vector.bn_aggr(mv4[:, q, :], st6[:, :])
            rstd4 = p0s.tile([128, 4], F32, tag="rstd4p0")
            nc.scalar.activation(rstd4[:, :], mv4[:, :, 1], AF.Sqrt, bias=eps_sb[:, :],
                                 scale=1.0)
            nc.vector.reciprocal(rstd4[:, :], rstd4[:, :])
            for q in range(4):
                nmu = p0s.tile([128, 1], F32, tag="nmu")
                nc.vector.tensor_scalar(out=nmu[:, :], in0=mv4[:, q, 0:1],
                                        scalar1=rstd4[:, q:q + 1],
                                        scalar2=-1.0, op0=OP.mult, op1=OP.mult)
                zt = p0.tile([128, D], BF16, tag="zt")
                nc.vector.tensor_scalar(out=zt[:, :], in0=xt4[:, q, :],
                                        scalar1=rstd4[:, q:q + 1],
                                        scalar2=nmu[:, :], op0=OP.mult, op1=OP.add)
                nc.sync.dma_start_transpose(zT[:, :, t0 + q * 128:t0 + (q + 1) * 128],
                                            zt[:, :])
            ps = p1p.tile([128, 512], F32, tag="qkps")
            for kt in range(4):
                nc.tensor.matmul(ps[:, :], wqk_sb[:, kt, :], zT[:, kt, ch * 512:(ch + 1) * 512],
                                 start=(kt == 0), stop=(kt == 3))
            nc.scalar.activation(qkp[:, PAD + ch * 512:PAD + (ch + 1) * 512], ps[:, :],
                                 AF.Silu, bias=bqk_sb[:, :], scale=1.0)

        qkc = p1.tile([128, N], BF16, tag="qkc")
        if CONV_PE_QK:
            with tc.tile_pool(name="p1cp", bufs=2, space="PSUM") as p1cp:
                hp8 = _build_hp8(nc, p1, qkp)
                dg = _build_dg(nc, p1, diag_mask, dwqk_sb, 0)
                _conv_dr(nc, p1cp, qkp, hp8, dg, diag_mask, qkc, nc.scalar)
        else:
            qkp1 = p1.tile([128, 2 * PAD + N], BF16, tag="qkpad1")
            nc.scalar.activation(qkp1[:, 0:2 * PAD + N - 2], qkp[:, 1:2 * PAD + N - 1], AF.Copy)
            qscr = p1.tile([128, N], BF16, tag="qscr")
            _conv_dve(nc, qscr, qkp, qkp1, dwqk_sb, 0, qkc)
        qq = p1.tile([128, N], BF16, tag="qq")
        qkk = p1.tile([128, N], BF16, tag="qkk")
        lkk = p1.tile([128, N], BF16, tag="lkk")
        for i, dst in ((0, qq), (1, lq_sb), (2, qkk), (3, lkk)):
            nc.vector.tensor_scalar(out=dst[:, :], in0=qkc[:, :], scalar1=gb_sb[:, i:i + 1],
                                    scalar2=gb_sb[:, 4 + i:5 + i], op0=OP.mult, op1=OP.add)
        nc.sync.dma_start_transpose(lk_str[:, :, :], lkk[:, :])

        for g in range(NG):
            for jh in range(2):
                sp = p1p.tile([128, G], F32, tag="simps")
                nc.tensor.matmul(sp[:, :], qkk[:, g * G + jh * 128: g * G + jh * 128 + 128],
                                 qq[:, g * G:(g + 1) * G], start=True, stop=True)
                rel = p1.tile([128, G], BF16, tag="rel")
                nc.scalar.activation(rel[:, :], sp[:, :], AF.Relu)
                nc.vector.tensor_mul(attnT[:, g, jh, :], rel[:, :], rel[:, :])

    # P3: hidden + conv + spill + lin_kv/lin_ku
    spill_v = spill.ap().rearrange("(tt p) (q c4) -> p tt q c4", p=128, c4=512)
    with tc.tile_pool(name="p3w", bufs=4) as p3w, \
         tc.tile_pool(name="p3h", bufs=3) as p3h, \
         tc.tile_pool(name="p3", bufs=2) as p3, \
         tc.tile_pool(name="p3q", bufs=1) as p3q, \
         tc.tile_pool(name="p3p", bufs=2, space="PSUM") as p3p, \
         tc.tile_pool(name="p3c", bufs=2, space="PSUM") as p3c, \
         tc.tile_pool(name="p3lin", bufs=2, space="PSUM") as p3lin:
        state = {"strips4": None}

        def produce(hc):
            wt = p3w.tile([128, 4, 128], BF16, tag="wt")
            nc.sync.dma_start(wt[:, :, :], wh[:, :, hc * 128:(hc + 1) * 128])
            hpad = p3h.tile([128, 2 * PAD + N], BF16, tag="hpad")
            nc.vector.memset(hpad[:, 0:PAD], 0.0)
            nc.vector.memset(hpad[:, PAD + N:], 0.0)
            for cp2 in range(4):
                c0 = 2 * cp2
                ps = p3p.tile([128, 1024], F32, tag="hps")
                for kt in range(4):
                    nc.tensor.matmul(ps[:, 0:512], wt[:, kt, :],
                                     zT[:, kt, c0 * 512:(c0 + 1) * 512],
                                     start=(kt == 0), stop=(kt == 3))
                    nc.tensor.matmul(ps[:, 512:1024], wt[:, kt, :],
                                     zT[:, kt, (c0 + 1) * 512:(c0 + 2) * 512],
                                     start=(kt == 0), stop=(kt == 3))
                nc.scalar.activation(hpad[:, PAD + c0 * 512:PAD + (c0 + 2) * 512], ps[:, :],
                                     AF.Silu, bias=bh_sb[:, hc:hc + 1], scale=1.0)
            return hpad

        def convpost(hc, hpad):
            if hc % 4 == 0:
                s4_new = p3q.tile([128, NT, 4, 128], BF16, tag="strips4")
                state["strips4"] = s4_new
            strips4 = state["strips4"]
            acc = p3.tile([128, N], BF16, tag="acc")
            if hc in CONV_PE_HID:
                hp8 = _build_hp8(nc, p3, hpad)
                dg = _build_dg(nc, p3, diag_mask, dwh_sb, hc)
                _conv_dr(nc, p3c, hpad, hp8, dg, diag_mask, acc, nc.scalar)
            else:
                hpad1 = p3q.tile([128, 2 * PAD + N], BF16, tag="hpad1")
                nc.scalar.activation(hpad1[:, 0:2 * PAD + N - 2], hpad[:, 1:2 * PAD + N - 1],
                                     AF.Copy)
                scr = p3q.tile([128, N], BF16, tag="convscr")
                _conv_dve(nc, scr, hpad, hpad1, dwh_sb, hc, acc)
            nc.sync.dma_start_transpose(strips4[:, :, hc % 4, :], acc[:, :])
            if hc % 4 == 3:
                q = hc // 4
                nc.sync.dma_start(spill_v[:, :, q, :], strips4[:, :, :, :])
                dst = linkv_sb if hc < 8 else linku_sb
                col = (q % 2) * 512
                lps = p3lin.tile([128, 512], F32, tag="linps")
                for tt in range(NT):
                    nc.tensor.matmul(
                        lps[:, :], lk_str[:, tt, :],
                        strips4[:, tt, :, :].rearrange("p a c -> p (a c)"),
                        start=(tt == 0), stop=(tt == NT - 1))
                nc.scalar.activation(dst[:, col:col + 512], lps[:, :], AF.Copy)

        pending = []
        for hc in range(16):
            pending.append((hc, produce(hc)))
            if len(pending) > 2:
                convpost(*pending.pop(0))
        for item in pending:
            convpost(*item)
    p03.release()

    # P4: attention + gating + LN_o ; z_o kept SBUF-resident (transposed)
    pz = tc.alloc_tile_pool(name="pz", bufs=1)
    z_oT = pz.tile([128, 8, N], BF16)
    with tc.tile_pool(name="p4", bufs=2) as p4, \
         tc.tile_pool(name="p4g", bufs=2) as p4g, \
         tc.tile_pool(name="p4s", bufs=8) as p4s, \
         tc.tile_pool(name="p4p", bufs=2, space="PSUM") as p4p:
        for gpair in range(NG // 2):
            # batch = 2 groups x 2 it-tiles = 4 token tiles; deferred sqrt
            gos = []
            mv4 = p4s.tile([128, 4, 2], F32, tag="mv4")
            vgs, ugs = {}, {}
            for half_g in range(2):
                g = gpair * 2 + half_g
                vg, ug = [], []
                for jh in range(2):
                    vt = p4.tile([128, E2], BF16, tag=f"vg{half_g}{jh}")
                    nc.sync.dma_start(vt[:, :], spill[g * G + jh * 128: g * G + jh * 128 + 128, 0:E2])
                    ut = p4.tile([128, E2], BF16, tag=f"ug{half_g}{jh}")
                    nc.sync.dma_start(ut[:, :], spill[g * G + jh * 128: g * G + jh * 128 + 128, E2:H])
                    vg.append(vt)
                    ug.append(ut)
                vgs[g], ugs[g] = vg, ug
            for j in range(4):
                g = gpair * 2 + j // 2
                it = j % 2
                vg, ug = vgs[g], ugs[g]
                ap_ = p4p.tile([128, 2 * E2], F32, tag="attps")
                islice = slice(g * G + it * 128, g * G + it * 128 + 128)
                for half, (grp, lin) in enumerate(((vg, linkv_sb), (ug, linku_sb))):
                    base = half * E2
                    for e in range(2):
                        for jh in range(2):
                            nc.tensor.matmul(ap_[:, base + e * 512:base + (e + 1) * 512],
                                             attnT[:, g, jh, it * 128:it * 128 + 128],
                                             grp[jh][:, e * 512:(e + 1) * 512],
                                             start=(jh == 0), stop=False)
                        nc.tensor.matmul(ap_[:, base + e * 512:base + (e + 1) * 512],
                                         lq_sb[:, islice], lin[:, e * 512:(e + 1) * 512],
                                         start=False, stop=True)
                avau = p4.tile([128, 2 * E2], BF16, tag="avau")
                nc.scalar.activation(avau[:, :], ap_[:, :], AF.Copy)
                t1 = p4.tile([128, E2], BF16, tag="t1")
                nc.vector.tensor_mul(t1[:, :], ug[it][:, :], avau[:, 0:E2])
                sg = p4.tile([128, E2], BF16, tag="sg")
                nc.scalar.activation(sg[:, :], t1[:, :], AF.Sigmoid)
                t2 = p4.tile([128, E2], BF16, tag="t2")
                nc.gpsimd.tensor_mul(t2[:, :], vg[it][:, :], avau[:, E2:2 * E2])
                go = p4g.tile([128, E2], BF16, tag=f"go{j}")
                nc.vector.scalar_tensor_tensor(out=go[:, :], in0=t2[:, :], scalar=1.0,
                                               in1=sg[:, :], op0=OP.mult, op1=OP.mult)
                st6g = p4s.tile([128, 2, 6], F32, tag="st6g")
                nc.vector.bn_stats(st6g[:, 0, :], go[:, 0:E2 // 2])
                nc.vector.bn_stats(st6g[:, 1, :], go[:, E2 // 2:E2])
                nc.vector.bn_aggr(mv4[:, j, :], st6g[:, :, :].rearrange("p a b -> p (a b)"))
                gos.append(go)
            # batched rsqrt over the 4 tiles (one Sqrt act -> one table load)
            rstd4 = p4s.tile([128, 4], F32, tag="rstd4")
            nc.scalar.activation(rstd4[:, :], mv4[:, :, 1], AF.Sqrt, bias=eps_sb[:, :], scale=1.0)
            nc.vector.reciprocal(rstd4[:, :], rstd4[:, :])
            for j in range(4):
                g = gpair * 2 + j // 2
                it = j % 2
                nmu = p4s.tile([128, 1], F32, tag="nmu4")
                nc.vector.tensor_scalar(out=nmu[:, :], in0=mv4[:, j, 0:1],
                                        scalar1=rstd4[:, j:j + 1],
                                        scalar2=-1.0, op0=OP.mult, op1=OP.mult)
                zo = p4.tile([128, E2], BF16, tag="zo")
                nc.vector.tensor_scalar(out=zo[:, :], in0=gos[j][:, :], scalar1=rstd4[:, j:j + 1],
                                        scalar2=nmu[:, :], op0=OP.mult, op1=OP.add)
                tti = g * 2 + it
                nc.sync.dma_start_transpose(z_oT[:, :, tti * 128:(tti + 1) * 128], zo[:, :])
    qs.release()

    # P5: output FFConvM reading z_oT from SBUF
    pvo = tc.alloc_tile_pool(name="pvo", bufs=1, side="right")
    vo_big = pvo.tile([128, NT, 4, 128], BF16)
    with tc.tile_pool(name="p5", bufs=2) as p5, \
         tc.tile_pool(name="p5q", bufs=1) as p5q, \
         tc.tile_pool(name="p5p", bufs=2, space="PSUM") as p5p, \
         tc.tile_pool(name="p5c", bufs=2, space="PSUM") as p5c:

        def produce5(oc):
            hpad = p5.tile([128, 2 * PAD + N], BF16, tag="hpad5")
            nc.vector.memset(hpad[:, 0:PAD], 0.0)
            nc.vector.memset(hpad[:, PAD + N:], 0.0)
            for cp2 in range(4):
                c0 = 2 * cp2
                ps = p5p.tile([128, 1024], F32, tag="ops")
                for kt in range(8):
                    nc.tensor.matmul(ps[:, 0:512], wo_sb[:, kt, oc * 128:(oc + 1) * 128],
                                     z_oT[:, kt, c0 * 512:(c0 + 1) * 512],
                                     start=(kt == 0), stop=(kt == 7))
                    nc.tensor.matmul(ps[:, 512:1024], wo_sb[:, kt, oc * 128:(oc + 1) * 128],
                                     z_oT[:, kt, (c0 + 1) * 512:(c0 + 2) * 512],
                                     start=(kt == 0), stop=(kt == 7))
                nc.scalar.activation(hpad[:, PAD + c0 * 512:PAD + (c0 + 2) * 512], ps[:, :],
                                     AF.Silu, bias=bo_sb[:, oc:oc + 1], scale=1.0)
            return hpad

        def convpost5(oc, hpad):
            acc = p5.tile([128, N], BF16, tag="acc5")
            if oc in CONV_PE_O:
                hp8 = _build_hp8(nc, p5, hpad)
                dg = _build_dg(nc, p5, diag_mask, dwo_sb, oc)
                _conv_dr(nc, p5c, hpad, hp8, dg, diag_mask, acc, nc.scalar)
            else:
                hpad1 = p5q.tile([128, 2 * PAD + N], BF16, tag="hpad51")
                nc.scalar.activation(hpad1[:, 0:2 * PAD + N - 2], hpad[:, 1:2 * PAD + N - 1],
                                     AF.Copy)
                scr = p5q.tile([128, N], BF16, tag="convscr5")
                _conv_dve(nc, scr, hpad, hpad1, dwo_sb, oc, acc)
            nc.sync.dma_start_transpose(vo_big[:, :, oc, :], acc[:, :])

        prev = None
        for oc in range(4):
            hp = produce5(oc)
            if prev is not None:
                convpost5(*prev)
            prev = (oc, hp)
        convpost5(*prev)
    pz.release()

    # P6: residual, 4 token-tiles per DMA; x prefetched during P5, adds split
    # across the vector and gpsimd engines to shorten the tail.
    with tc.tile_pool(name="p6x", bufs=1) as p6x, \
         tc.tile_pool(name="p6", bufs=4) as p6:
        xts = []
        for c in range(NT // 4):
            t0 = c * 512
            xt = p6x.tile([128, 4, D], F32, tag=f"xt6{c}")
            nc.sync.dma_start(xt[:, :, :],
                              x[t0:t0 + 512, :].rearrange("(q p) d -> p q d", p=128))
            xts.append(xt)
        for c in range(NT // 4):
            t0 = c * 512
            of = p6.tile([128, 4, D], F32, tag="of")
            eng = nc.vector if c % 2 == 0 else nc.gpsimd
            eng.tensor_add(
                of[:, :, :].rearrange("p q d -> p (q d)"),
                xts[c][:, :, :].rearrange("p q d -> p (q d)"),
                vo_big[:, 4 * c:4 * c + 4, :, :].rearrange("p q a c -> p (q a c)"))
            nc.sync.dma_start(out[t0:t0 + 512, :].rearrange("(q p) d -> p q d", p=128),
                              of[:, :, :])
    pvo.release()
    consts.release()


def _build_nc():
    nc = bacc.Bacc("TRN2", target_bir_lowering=False, debug=False)
    x = nc.dram_tensor("x", [N, D], F32, kind="ExternalInput")
    wh = nc.dram_tensor("wh", [128, 4, H], BF16, kind="ExternalInput")
    wqk = nc.dram_tensor("wqk", [128, 4, QK], BF16, kind="ExternalInput")
    wo = nc.dram_tensor("wo", [128, 8, D], BF16, kind="ExternalInput")
    bh = nc.dram_tensor("bh", [128, 16], F32, kind="ExternalInput")
    bqk = nc.dram_tensor("bqk", [128, 1], F32, kind="ExternalInput")
    bo = nc.dram_tensor("bo", [128, 4], F32, kind="ExternalInput")
    dwh = nc.dram_tensor("dwh", [128, 16, KTAPS], F32, kind="ExternalInput")
    dwqk = nc.dram_tensor("dwqk", [128, 1, KTAPS], F32, kind="ExternalInput")
    dwo = nc.dram_tensor("dwo", [128, 4, KTAPS], F32, kind="ExternalInput")
    gb = nc.dram_tensor("gb", [128, 8], F32, kind="ExternalInput")
    out = nc.dram_tensor("out", [N, D], F32, kind="ExternalOutput")
    spill = nc.dram_tensor("spill", [N, H], BF16)
    with tile.TileContext(nc) as tc:
        _emit(nc, tc, x, wh, wqk, wo, bh, bqk, bo, dwh, dwqk, dwo, gb, out, spill)
    nc.compile()
    return nc


def prep_inputs(inputs):
    f32 = np.float32
    bf = ml_dtypes.bfloat16
    W_h = np.asarray(inputs["W_h"], f32)
    W_qk = np.asarray(inputs["W_qk"], f32)
    W_o = np.asarray(inputs["W_o"], f32)
    whp = np.asarray(inputs["ln_h_g"], f32)[:, None] * W_h
    bhp = np.asarray(inputs["ln_h_b"], f32) @ W_h + np.asarray(inputs["b_h"], f32)
    wqkp = np.asarray(inputs["ln_qk_g"], f32)[:, None] * W_qk
    bqkp = np.asarray(inputs["ln_qk_b"], f32) @ W_qk + np.asarray(inputs["b_qk"], f32)
    wop = np.asarray(inputs["ln_o_g"], f32)[:, None] * W_o
    bop = np.asarray(inputs["ln_o_b"], f32) @ W_o + np.asarray(inputs["b_o"], f32)
    gamma = np.asarray(inputs["gamma"], f32).copy()
    beta = np.asarray(inputs["beta"], f32).copy()
    gamma[0] /= G
    beta[0] /= G
    gamma[3] /= N
    beta[3] /= N

    def lhsT(w, ktiles):
        return np.ascontiguousarray(w.reshape(ktiles, 128, -1).transpose(1, 0, 2)).astype(bf)

    def chan(v, ntiles):
        return np.ascontiguousarray(v.reshape(ntiles, 128).T).astype(f32)

    def dwl(dw, ntiles):
        return np.ascontiguousarray(
            dw.T.reshape(ntiles, 128, KTAPS).transpose(1, 0, 2)).astype(f32)

    return {
        "wh": lhsT(whp, 4), "wqk": lhsT(wqkp, 4), "wo": lhsT(wop, 8),
        "bh": chan(bhp, 16), "bqk": chan(bqkp, 1), "bo": chan(bop, 4),
        "dwh": dwl(np.asarray(inputs["dw_h"], f32), 16),
        "dwqk": dwl(np.asarray(inputs["dw_qk"], f32), 1),
        "dwo": dwl(np.asarray(inputs["dw_o"], f32), 4),
        "gb": np.concatenate([gamma.T, beta.T], axis=1).astype(f32),
    }


_NC = None


def get_nc():
    global _NC
    if _NC is None:
        _NC = _build_nc()
    return _NC


def make_in_maps(inputs):
    x = np.asarray(inputs["x"], np.float32)
    B = x.shape[0]
    prep = prep_inputs(inputs)
    return [{"x": np.ascontiguousarray(x[b]), **prep} for b in range(B)]


def kernel(**inputs):
    nc = get_nc()
    in_maps = make_in_maps(inputs)
    res = bass_utils.run_bass_kernel_spmd(nc, in_maps, core_ids=list(range(8)))
    out = np.stack([res.results[b]["out"] for b in range(8)], axis=0)
    return out.astype(np.float32)


# revision 25
# speedup vs baseline: 1.0916x; 1.0916x over previous
"""Self-contained TRN2 kernel for nn_FLASH_ShareA_FFConvM_FlashAttn.

kernel(**inputs) takes the full (unsharded) inputs from setup_inputs() and
returns the full (B, N, D) float32 output. Internally: data-parallel over the
batch — one batch sample per NeuronCore, 8 cores, no collectives.

v2: depthwise convs via fp8 DoubleRow diag matmuls on the PE; z_o kept
SBUF-resident (no DRAM roundtrip); batched Sqrt in the gating phase to avoid
activation-table thrash; shift copies on the scalar engine.
"""
import sys

if "/opt/trn_rl_repo" not in sys.path:
    sys.path.insert(0, "/opt/trn_rl_repo")

import numpy as np
import ml_dtypes
import concourse.bass as bass
import concourse.bacc as bacc
import concourse.mybir as mybir
import concourse.tile as tile
from concourse import bass_utils

F32 = mybir.dt.float32
BF16 = mybir.dt.bfloat16
FP8 = mybir.dt.float8e4
AF = mybir.ActivationFunctionType
OP = mybir.AluOpType
PM = mybir.MatmulPerfMode

N, D, H, QK, G = 4096, 512, 2048, 128, 256
NG = N // G
NT = N // 128
KTAPS = 17
PAD = 8
E2 = 2 * D
EPS = 1e-5
W8 = 2 * PAD + N + 16  # fp8 double-plane buffer width (4128, mult of 16)

# channel tiles of the depthwise convs on the PE (fp8 DoubleRow diag matmuls);
# the rest run on the vector engine. DVE tiles are scheduled early so no
# phase ends on a long vector-engine tail.
CONV_PE_HID = frozenset(range(16)) - {5, 9}
CONV_PE_QK = True
CONV_PE_O = frozenset({0, 2, 3})
# DoubleRow tap pairs (k, k+2) matching a +2-column plane-1 shift
PAIR_KS = [0, 1, 4, 5, 8, 9, 12, 13]


def _conv_dve(nc, scratch, hpad, hpad1, dw_sb, dwi, acc):
    """acc = h + conv(h) via tensor_scalar products (4x) + tensor_tensor adds (2x)."""
    for k in range(KTAPS):
        s = k - PAD
        if s % 2 == 0:
            src, off = hpad, PAD + s
        else:
            src, off = hpad1, PAD - 1 + s
        if k == 0:
            nc.vector.scalar_tensor_tensor(
                out=acc[:, :], in0=src[:, off:off + N], scalar=dw_sb[:, dwi, 0:1],
                in1=hpad[:, PAD:PAD + N], op0=OP.mult, op1=OP.add)
        else:
            nc.vector.tensor_scalar(out=scratch[:, :], in0=src[:, off:off + N],
                                    scalar1=dw_sb[:, dwi, k:k + 1], scalar2=None,
                                    op0=OP.mult)
            nc.vector.tensor_add(acc[:, :], acc[:, :], scratch[:, :])


def _build_hp8(nc, pool, hpad):
    """fp8 double-plane buffer: plane0 = fp8(hpad), plane1 = plane0 shifted +2."""
    hp8 = pool.tile([128, 2, W8], FP8, tag="hp8")
    nc.scalar.activation(hp8[:, 0, 0:2 * PAD + N], hpad[:, :], AF.Copy)
    nc.vector.memset(hp8[:, 0, 2 * PAD + N:W8], 0.0)
    nc.sync.dma_start(hp8[:, 1, 0:W8 - 2], hp8[:, 0, 2:W8])
    nc.vector.memset(hp8[:, 1, W8 - 2:W8], 0.0)
    return hp8


def _build_dg(nc, pool, diag_mask, dw_sb, dwi):
    """fp8 diag weight pairs [128, 9, 2, 128] for DoubleRow conv."""
    dg = pool.tile([128, 9, 2, 128], FP8, tag="dg")
    for j, k in enumerate(PAIR_KS):
        nc.vector.tensor_scalar(out=dg[:, j, 0, :], in0=diag_mask[:, :],
                                scalar1=dw_sb[:, dwi, k:k + 1], scalar2=None, op0=OP.mult)
        nc.vector.tensor_scalar(out=dg[:, j, 1, :], in0=diag_mask[:, :],
                                scalar1=dw_sb[:, dwi, k + 2:k + 3], scalar2=None, op0=OP.mult)
    nc.vector.tensor_scalar(out=dg[:, 8, 0, :], in0=diag_mask[:, :],
                            scalar1=dw_sb[:, dwi, 16:17], scalar2=None, op0=OP.mult)
    nc.vector.memset(dg[:, 8, 1, :], 0.0)
    return dg


def _conv_dr(nc, psum_pool, hpad, hp8, dg, diag_mask, acc, evac):
    """acc = h + conv(h): bf16 identity + 9 fp8 DoubleRow diag pair-matmuls."""
    for half in range(4):
        ps0 = psum_pool.tile([128, 512], F32, tag="convdr")
        ps1 = psum_pool.tile([128, 512], F32, tag="convdr")
        ps = [ps0, ps1]
        bases = [PAD + (2 * half + i) * 512 for i in range(2)]
        for i in range(2):
            nc.tensor.matmul(ps[i][:, :], diag_mask[:, :], hpad[:, bases[i]:bases[i] + 512],
                             start=True, stop=False, skip_group_check=True)
        for j, k in enumerate(PAIR_KS + [16]):
            s = k - PAD
            last = j == 8
            for i in range(2):
                nc.tensor.matmul(ps[i][:, :], dg[:, j, :, :],
                                 hp8[:, :, bases[i] + s:bases[i] + s + 512],
                                 start=False, stop=last, perf_mode=PM.DoubleRow,
                                 skip_group_check=True)
        for i in range(2):
            c = 2 * half + i
            evac.activation(acc[:, c * 512:(c + 1) * 512], ps[i][:, :], AF.Copy)


def _emit(nc, tc, x, wh, wqk, wo, bh, bqk, bo, dwh, dwqk, dwo, gb, out, spill):
    consts = tc.alloc_tile_pool(name="consts", bufs=1)
    wqk_sb = consts.tile([128, 4, QK], BF16)
    nc.sync.dma_start(wqk_sb[:, :, :], wqk.ap())
    wo_sb = consts.tile([128, 8, D], BF16)
    nc.sync.dma_start(wo_sb[:, :, :], wo.ap())
    bh_sb = consts.tile([128, 16], F32)
    nc.sync.dma_start(bh_sb[:, :], bh.ap())
    bqk_sb = consts.tile([128, 1], F32)
    nc.sync.dma_start(bqk_sb[:, :], bqk.ap())
    bo_sb = consts.tile([128, 4], F32)
    nc.sync.dma_start(bo_sb[:, :], bo.ap())
    dwh_sb = consts.tile([128, 16, KTAPS], F32)
    nc.sync.dma_start(dwh_sb[:, :, :], dwh.ap())
    dwqk_sb = consts.tile([128, 1, KTAPS], F32)
    nc.sync.dma_start(dwqk_sb[:, :, :], dwqk.ap())
    dwo_sb = consts.tile([128, 4, KTAPS], F32)
    nc.sync.dma_start(dwo_sb[:, :, :], dwo.ap())
    gb_sb = consts.tile([128, 8], F32)
    nc.sync.dma_start(gb_sb[:, :], gb.ap())
    eps_sb = consts.tile([128, 1], F32)
    nc.vector.memset(eps_sb[:, :], EPS)

    iota_row = consts.tile([128, 128], F32)
    nc.gpsimd.iota(iota_row[:, :], pattern=[[1, 128]], base=0, channel_multiplier=0,
                   allow_small_or_imprecise_dtypes=True)
    iota_p = consts.tile([128, 1], F32)
    nc.gpsimd.iota(iota_p[:, :], pattern=[[0, 1]], base=0, channel_multiplier=1,
                   allow_small_or_imprecise_dtypes=True)
    diag_mask = consts.tile([128, 128], BF16)
    nc.vector.tensor_scalar(out=diag_mask[:, :], in0=iota_row[:, :],
                            scalar1=iota_p[:, :], scalar2=None, op0=OP.is_equal)

    p03 = tc.alloc_tile_pool(name="p03", bufs=1)     # P0-P3: zT
    zT = p03.tile([128, 4, N], BF16)
    qs = tc.alloc_tile_pool(name="qside", bufs=1, side="right")    # P1-P4
    attnT = qs.tile([128, NG, 2, G], BF16)
    lq_sb = qs.tile([128, N], BF16)
    lk_str = qs.tile([128, NT, 128], BF16)
    linkv_sb = qs.tile([128, E2], BF16)
    linku_sb = qs.tile([128, E2], BF16)

    # P0 + P1 head: token-shifted LayerNorm, with the qk GEMM interleaved
    # per 512-token chunk (keeps the PE warm during the load/normalize phase).
    with tc.tile_pool(name="p0", bufs=3) as p0, \
         tc.tile_pool(name="p0s", bufs=8) as p0s, \
         tc.tile_pool(name="p1", bufs=1) as p1, \
         tc.tile_pool(name="p1p", bufs=2, space="PSUM") as p1p:
        qkp = p1.tile([128, 2 * PAD + N], BF16, tag="qkpad")
        qkpre = p1.tile([128, N], BF16, tag="qkpre")
        nc.vector.memset(qkp[:, 0:PAD], 0.0)
        nc.vector.memset(qkp[:, PAD + N:], 0.0)
        for ch in range(8):
            t0 = ch * 512
            xt4 = p0.tile([128, 4, D], F32, tag="xt4")
            if ch == 0:
                nc.vector.memset(xt4[0:1, 0, 0:D // 2], 0.0)
                nc.sync.dma_start(xt4[1:128, 0, 0:D // 2], x[0:127, 0:D // 2])
                nc.sync.dma_start(
                    xt4[:, 1:4, 0:D // 2],
                    x[127:511, 0:D // 2].rearrange("(q p) d -> p q d", p=128))
            else:
                nc.sync.dma_start(
                    xt4[:, :, 0:D // 2],
                    x[t0 - 1:t0 + 511, 0:D // 2].rearrange("(q p) d -> p q d", p=128))
            nc.sync.dma_start(
                xt4[:, :, D // 2:D],
                x[t0:t0 + 512, D // 2:D].rearrange("(q p) d -> p q d", p=128))
            mv4 = p0s.tile([128, 4, 2], F32, tag="mv4p0")
            for q in range(4):
                st6 = p0s.tile([128, 6], F32, tag="st6")
                nc.vector.bn_stats(st6[:, :], xt4[:, q, :])
                nc.vector.bn_aggr(mv4[:, q, :], st6[:, :])
            rstd4 = p0s.tile([128, 4], F32, tag="rstd4p0")
            nc.scalar.activation(rstd4[:, :], mv4[:, :, 1], AF.Sqrt, bias=eps_sb[:, :],
                                 scale=1.0)
            nc.vector.reciprocal(rstd4[:, :], rstd4[:, :])
            for q in range(4):
                nmu = p0s.tile([128, 1], F32, tag="nmu")
                nc.vector.tensor_scalar(out=nmu[:, :], in0=mv4[:, q, 0:1],
                                        scalar1=rstd4[:, q:q + 1],
                                        scalar2=-1.0, op0=OP.mult, op1=OP.mult)
                zt = p0.tile([128, D], BF16, tag="zt")
                nc.vector.tensor_scalar(out=zt[:, :], in0=xt4[:, q, :],
                                        scalar1=rstd4[:, q:q + 1],
                                        scalar2=nmu[:, :], op0=OP.mult, op1=OP.add)
                nc.sync.dma_start_transpose(zT[:, :, t0 + q * 128:t0 + (q + 1) * 128],
                                            zt[:, :])
            ps = p1p.tile([128, 512], F32, tag="qkps")
            for kt in range(4):
                nc.tensor.matmul(ps[:, :], wqk_sb[:, kt, :], zT[:, kt, ch * 512:(ch + 1) * 512],
                                 start=(kt == 0), stop=(kt == 3))
            # plain Copy evac (Copy lives in every act table, so P0's scalar
            # engine never leaves the Sqrt table); SiLU applied once below.
            nc.scalar.activation(qkpre[:, ch * 512:(ch + 1) * 512], ps[:, :], AF.Copy)
        nc.scalar.activation(qkp[:, PAD:PAD + N], qkpre[:, :], AF.Silu,
                             bias=bqk_sb[:, :], scale=1.0)

        qkc = p1.tile([128, N], BF16, tag="qkc")
        if CONV_PE_QK:
            with tc.tile_pool(name="p1cp", bufs=2, space="PSUM") as p1cp:
                hp8 = _build_hp8(nc, p1, qkp)
                dg = _build_dg(nc, p1, diag_mask, dwqk_sb, 0)
                _conv_dr(nc, p1cp, qkp, hp8, dg, diag_mask, qkc, nc.scalar)
        else:
            qkp1 = p1.tile([128, 2 * PAD + N], BF16, tag="qkpad1")
            nc.scalar.activation(qkp1[:, 0:2 * PAD + N - 2], qkp[:, 1:2 * PAD + N - 1], AF.Copy)
            qscr = p1.tile([128, N], BF16, tag="qscr")
            _conv_dve(nc, qscr, qkp, qkp1, dwqk_sb, 0, qkc)
        qq = p1.tile([128, N], BF16, tag="qq")
        qkk = p1.tile([128, N], BF16, tag="qkk")
        lkk = p1.tile([128, N], BF16, tag="lkk")
        for i, dst in ((0, qq), (1, lq_sb), (2, qkk), (3, lkk)):
            nc.vector.tensor_scalar(out=dst[:, :], in0=qkc[:, :], scalar1=gb_sb[:, i:i + 1],
                                    scalar2=gb_sb[:, 4 + i:5 + i], op0=OP.mult, op1=OP.add)
        nc.sync.dma_start_transpose(lk_str[:, :, :], lkk[:, :])

        for g in range(NG):
            for jh in range(2):
                sp = p1p.tile([128, G], F32, tag="simps")
                nc.tensor.matmul(sp[:, :], qkk[:, g * G + jh * 128: g * G + jh * 128 + 128],
                                 qq[:, g * G:(g + 1) * G], start=True, stop=True)
                rel = p1.tile([128, G], BF16, tag="rel")
                nc.scalar.activation(rel[:, :], sp[:, :], AF.Relu)
                nc.vector.tensor_mul(attnT[:, g, jh, :], rel[:, :], rel[:, :])

    # P3: hidden + conv + spill + lin_kv/lin_ku
    spill_v = spill.ap().rearrange("(tt p) (q c4) -> p tt q c4", p=128, c4=512)
    with tc.tile_pool(name="p3w", bufs=4) as p3w, \
         tc.tile_pool(name="p3h", bufs=3) as p3h, \
         tc.tile_pool(name="p3", bufs=2) as p3, \
         tc.tile_pool(name="p3q", bufs=1) as p3q, \
         tc.tile_pool(name="p3p", bufs=2, space="PSUM") as p3p, \
         tc.tile_pool(name="p3c", bufs=2, space="PSUM") as p3c, \
         tc.tile_pool(name="p3lin", bufs=2, space="PSUM") as p3lin:
        state = {"strips4": None}

        def produce(hc):
            wt = p3w.tile([128, 4, 128], BF16, tag="wt")
            nc.sync.dma_start(wt[:, :, :], wh[:, :, hc * 128:(hc + 1) * 128])
            hpad = p3h.tile([128, 2 * PAD + N], BF16, tag="hpad")
            nc.vector.memset(hpad[:, 0:PAD], 0.0)
            nc.vector.memset(hpad[:, PAD + N:], 0.0)
            for cp2 in range(4):
                c0 = 2 * cp2
                ps = p3p.tile([128, 1024], F32, tag="hps")
                for kt in range(4):
                    nc.tensor.matmul(ps[:, 0:512], wt[:, kt, :],
                                     zT[:, kt, c0 * 512:(c0 + 1) * 512],
                                     start=(kt == 0), stop=(kt == 3))
                    nc.tensor.matmul(ps[:, 512:1024], wt[:, kt, :],
                                     zT[:, kt, (c0 + 1) * 512:(c0 + 2) * 512],
                                     start=(kt == 0), stop=(kt == 3))
                nc.scalar.activation(hpad[:, PAD + c0 * 512:PAD + (c0 + 2) * 512], ps[:, :],
                                     AF.Silu, bias=bh_sb[:, hc:hc + 1], scale=1.0)
            return hpad

        def convpost(hc, hpad):
            if hc % 4 == 0:
                s4_new = p3q.tile([128, NT, 4, 128], BF16, tag="strips4")
                state["strips4"] = s4_new
            strips4 = state["strips4"]
            acc = p3.tile([128, N], BF16, tag="acc")
            if hc in CONV_PE_HID:
                hp8 = _build_hp8(nc, p3, hpad)
                dg = _build_dg(nc, p3, diag_mask, dwh_sb, hc)
                _conv_dr(nc, p3c, hpad, hp8, dg, diag_mask, acc, nc.scalar)
            else:
                hpad1 = p3q.tile([128, 2 * PAD + N], BF16, tag="hpad1")
                nc.scalar.activation(hpad1[:, 0:2 * PAD + N - 2], hpad[:, 1:2 * PAD + N - 1],
                                     AF.Copy)
                scr = p3q.tile([128, N], BF16, tag="convscr")
                _conv_dve(nc, scr, hpad, hpad1, dwh_sb, hc, acc)
            nc.sync.dma_start_transpose(strips4[:, :, hc % 4, :], acc[:, :])
            if hc % 4 == 3:
                q = hc // 4
                nc.sync.dma_start(spill_v[:, :, q, :], strips4[:, :, :, :])
                dst = linkv_sb if hc < 8 else linku_sb
                col = (q % 2) * 512
                lps = p3lin.tile([128, 512], F32, tag="linps")
                for tt in range(NT):
                    nc.tensor.matmul(
                        lps[:, :], lk_str[:, tt, :],
                        strips4[:, tt, :, :].rearrange("p a c -> p (a c)"),
                        start=(tt == 0), stop=(tt == NT - 1))
                nc.scalar.activation(dst[:, col:col + 512], lps[:, :], AF.Copy)

        pending = []
        for hc in range(16):
            pending.append((hc, produce(hc)))
            if len(pending) > 2:
                convpost(*pending.pop(0))
        for item in pending:
            convpost(*item)
    p03.release()

    # P4: attention + gating + LN_o ; z_o kept SBUF-resident (transposed)
    pz = tc.alloc_tile_pool(name="pz", bufs=1)
    z_oT = pz.tile([128, 8, N], BF16)
    with tc.tile_pool(name="p4", bufs=2) as p4, \
         tc.tile_pool(name="p4a", bufs=1) as p4a, \
         tc.tile_pool(name="p4g", bufs=2) as p4g, \
         tc.tile_pool(name="p4s", bufs=8) as p4s, \
         tc.tile_pool(name="p4p", bufs=2, space="PSUM") as p4p:
        for gpair in range(NG // 2):
            # batch = 2 groups x 2 it-tiles = 4 token tiles; deferred sqrt
            gos = []
            mv4 = p4s.tile([128, 4, 2], F32, tag="mv4")
            vgs, ugs = {}, {}
            for half_g in range(2):
                g = gpair * 2 + half_g
                vg, ug = [], []
                for jh in range(2):
                    vt = p4.tile([128, E2], BF16, tag=f"vg{half_g}{jh}")
                    nc.sync.dma_start(vt[:, :], spill[g * G + jh * 128: g * G + jh * 128 + 128, 0:E2])
                    ut = p4.tile([128, E2], BF16, tag=f"ug{half_g}{jh}")
                    nc.sync.dma_start(ut[:, :], spill[g * G + jh * 128: g * G + jh * 128 + 128, E2:H])
                    vg.append(vt)
                    ug.append(ut)
                vgs[g], ugs[g] = vg, ug
            # stage 1: all four attention matmul groups + PSUM evacs
            avaus = []
            for j in range(4):
                g = gpair * 2 + j // 2
                it = j % 2
                vg, ug = vgs[g], ugs[g]
                ap_ = p4p.tile([128, 2 * E2], F32, tag="attps")
                islice = slice(g * G + it * 128, g * G + it * 128 + 128)
                for half, (grp, lin) in enumerate(((vg, linkv_sb), (ug, linku_sb))):
                    base = half * E2
                    for e in range(2):
                        for jh in range(2):
                            nc.tensor.matmul(ap_[:, base + e * 512:base + (e + 1) * 512],
                                             attnT[:, g, jh, it * 128:it * 128 + 128],
                                             grp[jh][:, e * 512:(e + 1) * 512],
                                             start=(jh == 0), stop=False)
                        nc.tensor.matmul(ap_[:, base + e * 512:base + (e + 1) * 512],
                                         lq_sb[:, islice], lin[:, e * 512:(e + 1) * 512],
                                         start=False, stop=True)
                avau = p4a.tile([128, 2 * E2], BF16, tag=f"avau{j}")
                nc.scalar.activation(avau[:, :], ap_[:, :], AF.Copy)
                avaus.append(avau)
            # stage 2: gating elementwise, stats accumulate
            for j in range(4):
                g = gpair * 2 + j // 2
                it = j % 2
                vg, ug = vgs[g], ugs[g]
                avau = avaus[j]
                t1 = p4.tile([128, E2], BF16, tag="t1")
                nc.vector.tensor_mul(t1[:, :], ug[it][:, :], avau[:, 0:E2])
                sg = p4.tile([128, E2], BF16, tag="sg")
                nc.scalar.activation(sg[:, :], t1[:, :], AF.Sigmoid)
                t2 = p4.tile([128, E2], BF16, tag="t2")
                nc.gpsimd.tensor_mul(t2[:, :], vg[it][:, :], avau[:, E2:2 * E2])
                go = p4g.tile([128, E2], BF16, tag=f"go{j}")
                sumg = p4s.tile([128, 1], F32, tag="sumg")
                nc.vector.scalar_tensor_tensor(out=go[:, :], in0=t2[:, :], scalar=1.0,
                                               in1=sg[:, :], op0=OP.mult, op1=OP.mult,
                                               accum_out=sumg[:, :])
                g2 = p4.tile([128, E2], BF16, tag="g2")
                sumg2 = p4s.tile([128, 1], F32, tag="sumg2")
                nc.scalar.activation(g2[:, :], go[:, :], AF.Square, accum_out=sumg2[:, :])
                nc.vector.tensor_scalar_mul(mv4[:, j, 0:1], sumg[:, :], 1.0 / E2)
                mm = p4s.tile([128, 1], F32, tag="mm")
                nc.vector.tensor_scalar(out=mm[:, :], in0=mv4[:, j, 0:1], scalar1=mv4[:, j, 0:1],
                                        scalar2=-1.0, op0=OP.mult, op1=OP.mult)
                nc.vector.tensor_scalar(out=mv4[:, j, 1:2], in0=sumg2[:, :], scalar1=1.0 / E2,
                                        scalar2=mm[:, :], op0=OP.mult, op1=OP.add)
                gos.append(go)
            # batched rsqrt over the 4 tiles (one Sqrt act -> one table load)
            rstd4 = p4s.tile([128, 4], F32, tag="rstd4")
            nc.scalar.activation(rstd4[:, :], mv4[:, :, 1], AF.Sqrt, bias=eps_sb[:, :], scale=1.0)
            nc.vector.reciprocal(rstd4[:, :], rstd4[:, :])
            for j in range(4):
                g = gpair * 2 + j // 2
                it = j % 2
                nmu = p4s.tile([128, 1], F32, tag="nmu4")
                nc.vector.tensor_scalar(out=nmu[:, :], in0=mv4[:, j, 0:1],
                                        scalar1=rstd4[:, j:j + 1],
                                        scalar2=-1.0, op0=OP.mult, op1=OP.mult)
                zo = p4.tile([128, E2], BF16, tag="zo")
                nc.vector.tensor_scalar(out=zo[:, :], in0=gos[j][:, :], scalar1=rstd4[:, j:j + 1],
                                        scalar2=nmu[:, :], op0=OP.mult, op1=OP.add)
                tti = g * 2 + it
                nc.sync.dma_start_transpose(z_oT[:, :, tti * 128:(tti + 1) * 128], zo[:, :])
    qs.release()

    # P5: output FFConvM reading z_oT from SBUF
    pvo = tc.alloc_tile_pool(name="pvo", bufs=1, side="right")
    vo_big = pvo.tile([128, NT, 4, 128], BF16)
    with tc.tile_pool(name="p5", bufs=2) as p5, \
         tc.tile_pool(name="p5q", bufs=1) as p5q, \
         tc.tile_pool(name="p5p", bufs=2, space="PSUM") as p5p, \
         tc.tile_pool(name="p5c", bufs=2, space="PSUM") as p5c:

        def produce5(oc):
            hpad = p5.tile([128, 2 * PAD + N], BF16, tag="hpad5")
            nc.vector.memset(hpad[:, 0:PAD], 0.0)
            nc.vector.memset(hpad[:, PAD + N:], 0.0)
            for cp2 in range(4):
                c0 = 2 * cp2
                ps = p5p.tile([128, 1024], F32, tag="ops")
                for kt in range(8):
                    nc.tensor.matmul(ps[:, 0:512], wo_sb[:, kt, oc * 128:(oc + 1) * 128],
                                     z_oT[:, kt, c0 * 512:(c0 + 1) * 512],
                                     start=(kt == 0), stop=(kt == 7))
                    nc.tensor.matmul(ps[:, 512:1024], wo_sb[:, kt, oc * 128:(oc + 1) * 128],
                                     z_oT[:, kt, (c0 + 1) * 512:(c0 + 2) * 512],
                                     start=(kt == 0), stop=(kt == 7))
                nc.scalar.activation(hpad[:, PAD + c0 * 512:PAD + (c0 + 2) * 512], ps[:, :],
                                     AF.Silu, bias=bo_sb[:, oc:oc + 1], scale=1.0)
            return hpad

        def convpost5(oc, hpad):
            acc = p5.tile([128, N], BF16, tag="acc5")
            if oc in CONV_PE_O:
                hp8 = _build_hp8(nc, p5, hpad)
                dg = _build_dg(nc, p5, diag_mask, dwo_sb, oc)
                _conv_dr(nc, p5c, hpad, hp8, dg, diag_mask, acc, nc.scalar)
            else:
                hpad1 = p5q.tile([128, 2 * PAD + N], BF16, tag="hpad51")
                nc.scalar.activation(hpad1[:, 0:2 * PAD + N - 2], hpad[:, 1:2 * PAD + N - 1],
                                     AF.Copy)
                scr = p5q.tile([128, N], BF16, tag="convscr5")
                _conv_dve(nc, scr, hpad, hpad1, dwo_sb, oc, acc)
            nc.sync.dma_start_transpose(vo_big[:, :, oc, :], acc[:, :])

        prev = None
        for oc in range(4):
            hp = produce5(oc)
            if prev is not None:
                convpost5(*prev)
            prev = (oc, hp)
        convpost5(*prev)
    pz.release()

    # P6: residual, 4 token-tiles per DMA; x prefetched during P5, adds split
    # across the vector and gpsimd engines to shorten the tail.
    with tc.tile_pool(name="p6x", bufs=1) as p6x, \
         tc.tile_pool(name="p6", bufs=4) as p6:
        xts = []
        for c in range(NT // 4):
            t0 = c * 512
            xt = p6x.tile([128, 4, D], F32, tag=f"xt6{c}")
            nc.sync.dma_start(xt[:, :, :],
                              x[t0:t0 + 512, :].rearrange("(q p) d -> p q d", p=128))
            xts.append(xt)
        for c in range(NT // 4):
            t0 = c * 512
            of = p6.tile([128, 4, D], F32, tag="of")
            eng = nc.vector if c % 2 == 0 else nc.gpsimd
            eng.tensor_add(
                of[:, :, :].rearrange("p q d -> p (q d)"),
                xts[c][:, :, :].rearrange("p q d -> p (q d)"),
                vo_big[:, 4 * c:4 * c + 4, :, :].rearrange("p q a c -> p (q a c)"))
            nc.sync.dma_start(out[t0:t0 + 512, :].rearrange("(q p) d -> p q d", p=128),
                              of[:, :, :])
    pvo.release()
    consts.release()


def _build_nc():
    nc = bacc.Bacc("TRN2", target_bir_lowering=False, debug=False)
    x = nc.dram_tensor("x", [N, D], F32, kind="ExternalInput")
    wh = nc.dram_tensor("wh", [128, 4, H], BF16, kind="ExternalInput")
    wqk = nc.dram_tensor("wqk", [128, 4, QK], BF16, kind="ExternalInput")
    wo = nc.dram_tensor("wo", [128, 8, D], BF16, kind="ExternalInput")
    bh = nc.dram_tensor("bh", [128, 16], F32, kind="ExternalInput")
    bqk = nc.dram_tensor("bqk", [128, 1], F32, kind="ExternalInput")
    bo = nc.dram_tensor("bo", [128, 4], F32, kind="ExternalInput")
    dwh = nc.dram_tensor("dwh", [128, 16, KTAPS], F32, kind="ExternalInput")
    dwqk = nc.dram_tensor("dwqk", [128, 1, KTAPS], F32, kind="ExternalInput")
    dwo = nc.dram_tensor("dwo", [128, 4, KTAPS], F32, kind="ExternalInput")
    gb = nc.dram_tensor("gb", [128, 8], F32, kind="ExternalInput")
    out = nc.dram_tensor("out", [N, D], F32, kind="ExternalOutput")
    spill = nc.dram_tensor("spill", [N, H], BF16)
    with tile.TileContext(nc) as tc:
        _emit(nc, tc, x, wh, wqk, wo, bh, bqk, bo, dwh, dwqk, dwo, gb, out, spill)
    nc.compile()
    return nc


def prep_inputs(inputs):
    f32 = np.float32
    bf = ml_dtypes.bfloat16
    W_h = np.asarray(inputs["W_h"], f32)
    W_qk = np.asarray(inputs["W_qk"], f32)
    W_o = np.asarray(inputs["W_o"], f32)
    whp = np.asarray(inputs["ln_h_g"], f32)[:, None] * W_h
    bhp = np.asarray(inputs["ln_h_b"], f32) @ W_h + np.asarray(inputs["b_h"], f32)
    wqkp = np.asarray(inputs["ln_qk_g"], f32)[:, None] * W_qk
    bqkp = np.asarray(inputs["ln_qk_b"], f32) @ W_qk + np.asarray(inputs["b_qk"], f32)
    wop = np.asarray(inputs["ln_o_g"], f32)[:, None] * W_o
    bop = np.asarray(inputs["ln_o_b"], f32) @ W_o + np.asarray(inputs["b_o"], f32)
    gamma = np.asarray(inputs["gamma"], f32).copy()
    beta = np.asarray(inputs["beta"], f32).copy()
    gamma[0] /= G
    beta[0] /= G
    gamma[3] /= N
    beta[3] /= N

    def lhsT(w, ktiles):
        return np.ascontiguousarray(w.reshape(ktiles, 128, -1).transpose(1, 0, 2)).astype(bf)

    def chan(v, ntiles):
        return np.ascontiguousarray(v.reshape(ntiles, 128).T).astype(f32)

    def dwl(dw, ntiles):
        return np.ascontiguousarray(
            dw.T.reshape(ntiles, 128, KTAPS).transpose(1, 0, 2)).astype(f32)

    return {
        "wh": lhsT(whp, 4), "wqk": lhsT(wqkp, 4), "wo": lhsT(wop, 8),
        "bh": chan(bhp, 16), "bqk": chan(bqkp, 1), "bo": chan(bop, 4),
        "dwh": dwl(np.asarray(inputs["dw_h"], f32), 16),
        "dwqk": dwl(np.asarray(inputs["dw_qk"], f32), 1),
        "dwo": dwl(np.asarray(inputs["dw_o"], f32), 4),
        "gb": np.concatenate([gamma.T, beta.T], axis=1).astype(f32),
    }


_NC = None


def get_nc():
    global _NC
    if _NC is None:
        _NC = _build_nc()
    return _NC


def make_in_maps(inputs):
    x = np.asarray(inputs["x"], np.float32)
    B = x.shape[0]
    prep = prep_inputs(inputs)
    return [{"x": np.ascontiguousarray(x[b]), **prep} for b in range(B)]


def kernel(**inputs):
    nc = get_nc()
    in_maps = make_in_maps(inputs)
    res = bass_utils.run_bass_kernel_spmd(nc, in_maps, core_ids=list(range(8)))
    out = np.stack([res.results[b]["out"] for b in range(8)], axis=0)
    return out.astype(np.float32)


# revision 28
# speedup vs baseline: 1.1202x; 1.0262x over previous
"""Self-contained TRN2 kernel for nn_FLASH_ShareA_FFConvM_FlashAttn.

kernel(**inputs) takes the full (unsharded) inputs from setup_inputs() and
returns the full (B, N, D) float32 output. Internally: data-parallel over the
batch — one batch sample per NeuronCore, 8 cores, no collectives.

v2: depthwise convs via fp8 DoubleRow diag matmuls on the PE; z_o kept
SBUF-resident (no DRAM roundtrip); batched Sqrt in the gating phase to avoid
activation-table thrash; shift copies on the scalar engine.
"""
import sys

if "/opt/trn_rl_repo" not in sys.path:
    sys.path.insert(0, "/opt/trn_rl_repo")

import numpy as np
import ml_dtypes
import concourse.bass as bass
import concourse.bacc as bacc
import concourse.mybir as mybir
import concourse.tile as tile
from concourse import bass_utils

F32 = mybir.dt.float32
BF16 = mybir.dt.bfloat16
FP8 = mybir.dt.float8e4
AF = mybir.ActivationFunctionType
OP = mybir.AluOpType
PM = mybir.MatmulPerfMode

N, D, H, QK, G = 4096, 512, 2048, 128, 256
NG = N // G
NT = N // 128
KTAPS = 17
PAD = 8
E2 = 2 * D
EPS = 1e-5
W8 = 2 * PAD + N + 16  # fp8 double-plane buffer width (4128, mult of 16)

# channel tiles of the depthwise convs on the PE (fp8 DoubleRow diag matmuls);
# the rest run on the vector engine. DVE tiles are scheduled early so no
# phase ends on a long vector-engine tail.
CONV_PE_HID = frozenset(range(16)) - {5, 9}
CONV_PE_QK = True
CONV_PE_O = frozenset({0, 2, 3})
# DoubleRow tap pairs (k, k+2) matching a +2-column plane-1 shift
PAIR_KS = [0, 1, 4, 5, 8, 9, 12, 13]


def _conv_dve(nc, scratch, hpad, hpad1, dw_sb, dwi, acc):
    """acc = h + conv(h) via tensor_scalar products (4x) + tensor_tensor adds (2x)."""
    for k in range(KTAPS):
        s = k - PAD
        if s % 2 == 0:
            src, off = hpad, PAD + s
        else:
            src, off = hpad1, PAD - 1 + s
        if k == 0:
            nc.vector.scalar_tensor_tensor(
                out=acc[:, :], in0=src[:, off:off + N], scalar=dw_sb[:, dwi, 0:1],
                in1=hpad[:, PAD:PAD + N], op0=OP.mult, op1=OP.add)
        else:
            nc.vector.tensor_scalar(out=scratch[:, :], in0=src[:, off:off + N],
                                    scalar1=dw_sb[:, dwi, k:k + 1], scalar2=None,
                                    op0=OP.mult)
            nc.vector.tensor_add(acc[:, :], acc[:, :], scratch[:, :])


def _build_hp8(nc, pool, hpad):
    """fp8 double-plane buffer: plane0 = fp8(hpad), plane1 = plane0 shifted +2."""
    hp8 = pool.tile([128, 2, W8], FP8, tag="hp8")
    nc.scalar.activation(hp8[:, 0, 0:2 * PAD + N], hpad[:, :], AF.Copy)
    nc.vector.memset(hp8[:, 0, 2 * PAD + N:W8], 0.0)
    nc.sync.dma_start(hp8[:, 1, 0:W8 - 2], hp8[:, 0, 2:W8])
    nc.vector.memset(hp8[:, 1, W8 - 2:W8], 0.0)
    return hp8


def _build_dg(nc, pool, diag_mask, dw_sb, dwi):
    """fp8 diag weight pairs [128, 9, 2, 128] for DoubleRow conv."""
    dg = pool.tile([128, 9, 2, 128], FP8, tag="dg")
    for j, k in enumerate(PAIR_KS):
        nc.vector.tensor_scalar(out=dg[:, j, 0, :], in0=diag_mask[:, :],
                                scalar1=dw_sb[:, dwi, k:k + 1], scalar2=None, op0=OP.mult)
        nc.vector.tensor_scalar(out=dg[:, j, 1, :], in0=diag_mask[:, :],
                                scalar1=dw_sb[:, dwi, k + 2:k + 3], scalar2=None, op0=OP.mult)
    nc.vector.tensor_scalar(out=dg[:, 8, 0, :], in0=diag_mask[:, :],
                            scalar1=dw_sb[:, dwi, 16:17], scalar2=None, op0=OP.mult)
    nc.vector.memset(dg[:, 8, 1, :], 0.0)
    return dg


def _conv_dr(nc, psum_pool, hpad, hp8, dg, diag_mask, acc, evac):
    """acc = h + conv(h): 9 fp8 DoubleRow diag pair-matmuls; the identity tap
    (+h) rides the PSUM evacuation as a fused DVE scalar_tensor_tensor."""
    for half in range(4):
        ps0 = psum_pool.tile([128, 512], F32, tag="convdr")
        ps1 = psum_pool.tile([128, 512], F32, tag="convdr")
        ps = [ps0, ps1]
        bases = [PAD + (2 * half + i) * 512 for i in range(2)]
        for j, k in enumerate(PAIR_KS + [16]):
            s = k - PAD
            for i in range(2):
                nc.tensor.matmul(ps[i][:, :], dg[:, j, :, :],
                                 hp8[:, :, bases[i] + s:bases[i] + s + 512],
                                 start=(j == 0), stop=(j == 8), perf_mode=PM.DoubleRow,
                                 skip_group_check=True)
        for i in range(2):
            c = 2 * half + i
            nc.vector.scalar_tensor_tensor(
                out=acc[:, c * 512:(c + 1) * 512], in0=ps[i][:, :], scalar=1.0,
                in1=hpad[:, bases[i]:bases[i] + 512], op0=OP.mult, op1=OP.add)


def _emit(nc, tc, x, wh, wqk, wo, bh, bqk, bo, dwh, dwqk, dwo, gb, out, spill):
    consts = tc.alloc_tile_pool(name="consts", bufs=1)
    wqk_sb = consts.tile([128, 4, QK], BF16)
    nc.sync.dma_start(wqk_sb[:, :, :], wqk.ap())
    wo_sb = consts.tile([128, 8, D], BF16)
    nc.sync.dma_start(wo_sb[:, :, :], wo.ap())
    bh_sb = consts.tile([128, 16], F32)
    nc.sync.dma_start(bh_sb[:, :], bh.ap())
    bqk_sb = consts.tile([128, 1], F32)
    nc.sync.dma_start(bqk_sb[:, :], bqk.ap())
    bo_sb = consts.tile([128, 4], F32)
    nc.sync.dma_start(bo_sb[:, :], bo.ap())
    dwh_sb = consts.tile([128, 16, KTAPS], F32)
    nc.sync.dma_start(dwh_sb[:, :, :], dwh.ap())
    dwqk_sb = consts.tile([128, 1, KTAPS], F32)
    nc.sync.dma_start(dwqk_sb[:, :, :], dwqk.ap())
    dwo_sb = consts.tile([128, 4, KTAPS], F32)
    nc.sync.dma_start(dwo_sb[:, :, :], dwo.ap())
    gb_sb = consts.tile([128, 8], F32)
    nc.sync.dma_start(gb_sb[:, :], gb.ap())
    eps_sb = consts.tile([128, 1], F32)
    nc.vector.memset(eps_sb[:, :], EPS)

    iota_row = consts.tile([128, 128], F32)
    nc.gpsimd.iota(iota_row[:, :], pattern=[[1, 128]], base=0, channel_multiplier=0,
                   allow_small_or_imprecise_dtypes=True)
    iota_p = consts.tile([128, 1], F32)
    nc.gpsimd.iota(iota_p[:, :], pattern=[[0, 1]], base=0, channel_multiplier=1,
                   allow_small_or_imprecise_dtypes=True)
    diag_mask = consts.tile([128, 128], BF16)
    nc.vector.tensor_scalar(out=diag_mask[:, :], in0=iota_row[:, :],
                            scalar1=iota_p[:, :], scalar2=None, op0=OP.is_equal)

    p03 = tc.alloc_tile_pool(name="p03", bufs=1)     # P0-P3: zT
    zT = p03.tile([128, 4, N], BF16)
    qs = tc.alloc_tile_pool(name="qside", bufs=1, side="right")    # P1-P4
    attnT = qs.tile([128, NG, 2, G], BF16)
    lq_sb = qs.tile([128, N], BF16)
    lk_str = qs.tile([128, NT, 128], BF16)
    linkv_sb = qs.tile([128, E2], BF16)
    linku_sb = qs.tile([128, E2], BF16)

    # P0 + P1 head: token-shifted LayerNorm, with the qk GEMM interleaved
    # per 512-token chunk (keeps the PE warm during the load/normalize phase).
    with tc.tile_pool(name="p0", bufs=3) as p0, \
         tc.tile_pool(name="p0s", bufs=8) as p0s, \
         tc.tile_pool(name="p1", bufs=1) as p1, \
         tc.tile_pool(name="p1p", bufs=2, space="PSUM") as p1p:
        qkp = p1.tile([128, 2 * PAD + N], BF16, tag="qkpad")
        qkpre = p1.tile([128, N], BF16, tag="qkpre")
        nc.vector.memset(qkp[:, 0:PAD], 0.0)
        nc.vector.memset(qkp[:, PAD + N:], 0.0)
        for ch in range(8):
            t0 = ch * 512
            xt4 = p0.tile([128, 4, D], F32, tag="xt4")
            if ch == 0:
                nc.vector.memset(xt4[0:1, 0, 0:D // 2], 0.0)
                nc.sync.dma_start(xt4[1:128, 0, 0:D // 2], x[0:127, 0:D // 2])
                nc.sync.dma_start(
                    xt4[:, 1:4, 0:D // 2],
                    x[127:511, 0:D // 2].rearrange("(q p) d -> p q d", p=128))
            else:
                nc.sync.dma_start(
                    xt4[:, :, 0:D // 2],
                    x[t0 - 1:t0 + 511, 0:D // 2].rearrange("(q p) d -> p q d", p=128))
            nc.sync.dma_start(
                xt4[:, :, D // 2:D],
                x[t0:t0 + 512, D // 2:D].rearrange("(q p) d -> p q d", p=128))
            mv4 = p0s.tile([128, 4, 2], F32, tag="mv4p0")
            for q in range(4):
                st6 = p0s.tile([128, 6], F32, tag="st6")
                nc.vector.bn_stats(st6[:, :], xt4[:, q, :])
                nc.vector.bn_aggr(mv4[:, q, :], st6[:, :])
            rstd4 = p0s.tile([128, 4], F32, tag="rstd4p0")
            nc.scalar.activation(rstd4[:, :], mv4[:, :, 1], AF.Sqrt, bias=eps_sb[:, :],
                                 scale=1.0)
            nc.vector.reciprocal(rstd4[:, :], rstd4[:, :])
            for q in range(4):
                nmu = p0s.tile([128, 1], F32, tag="nmu")
                nc.vector.tensor_scalar(out=nmu[:, :], in0=mv4[:, q, 0:1],
                                        scalar1=rstd4[:, q:q + 1],
                                        scalar2=-1.0, op0=OP.mult, op1=OP.mult)
                zt = p0.tile([128, D], BF16, tag="zt")
                nc.vector.tensor_scalar(out=zt[:, :], in0=xt4[:, q, :],
                                        scalar1=rstd4[:, q:q + 1],
                                        scalar2=nmu[:, :], op0=OP.mult, op1=OP.add)
                nc.sync.dma_start_transpose(zT[:, :, t0 + q * 128:t0 + (q + 1) * 128],
                                            zt[:, :])
            ps = p1p.tile([128, 512], F32, tag="qkps")
            for kt in range(4):
                nc.tensor.matmul(ps[:, :], wqk_sb[:, kt, :], zT[:, kt, ch * 512:(ch + 1) * 512],
                                 start=(kt == 0), stop=(kt == 3))
            # plain Copy evac (Copy lives in every act table, so P0's scalar
            # engine never leaves the Sqrt table); SiLU applied once below.
            nc.scalar.activation(qkpre[:, ch * 512:(ch + 1) * 512], ps[:, :], AF.Copy)
        nc.scalar.activation(qkp[:, PAD:PAD + N], qkpre[:, :], AF.Silu,
                             bias=bqk_sb[:, :], scale=1.0)

        qkc = p1.tile([128, N], BF16, tag="qkc")
        if CONV_PE_QK:
            with tc.tile_pool(name="p1cp", bufs=2, space="PSUM") as p1cp:
                hp8 = _build_hp8(nc, p1, qkp)
                dg = _build_dg(nc, p1, diag_mask, dwqk_sb, 0)
                _conv_dr(nc, p1cp, qkp, hp8, dg, diag_mask, qkc, nc.scalar)
        else:
            qkp1 = p1.tile([128, 2 * PAD + N], BF16, tag="qkpad1")
            nc.scalar.activation(qkp1[:, 0:2 * PAD + N - 2], qkp[:, 1:2 * PAD + N - 1], AF.Copy)
            qscr = p1.tile([128, N], BF16, tag="qscr")
            _conv_dve(nc, qscr, qkp, qkp1, dwqk_sb, 0, qkc)
        qq = p1.tile([128, N], BF16, tag="qq")
        qkk = p1.tile([128, N], BF16, tag="qkk")
        lkk = p1.tile([128, N], BF16, tag="lkk")
        for i, dst in ((0, qq), (1, lq_sb), (2, qkk), (3, lkk)):
            nc.vector.tensor_scalar(out=dst[:, :], in0=qkc[:, :], scalar1=gb_sb[:, i:i + 1],
                                    scalar2=gb_sb[:, 4 + i:5 + i], op0=OP.mult, op1=OP.add)
        nc.sync.dma_start_transpose(lk_str[:, :, :], lkk[:, :])

        for g in range(NG):
            for jh in range(2):
                sp = p1p.tile([128, G], F32, tag="simps")
                nc.tensor.matmul(sp[:, :], qkk[:, g * G + jh * 128: g * G + jh * 128 + 128],
                                 qq[:, g * G:(g + 1) * G], start=True, stop=True)
                rel = p1.tile([128, G], BF16, tag="rel")
                nc.scalar.activation(rel[:, :], sp[:, :], AF.Relu)
                nc.vector.tensor_mul(attnT[:, g, jh, :], rel[:, :], rel[:, :])

    # P3: hidden + conv + spill + lin_kv/lin_ku
    spill_v = spill.ap().rearrange("(tt p) (q c4) -> p tt q c4", p=128, c4=512)
    with tc.tile_pool(name="p3w", bufs=4) as p3w, \
         tc.tile_pool(name="p3h", bufs=3) as p3h, \
         tc.tile_pool(name="p3", bufs=2) as p3, \
         tc.tile_pool(name="p3q", bufs=1) as p3q, \
         tc.tile_pool(name="p3p", bufs=2, space="PSUM") as p3p, \
         tc.tile_pool(name="p3c", bufs=2, space="PSUM") as p3c, \
         tc.tile_pool(name="p3lin", bufs=2, space="PSUM") as p3lin:
        state = {"strips4": None}

        def produce(hc):
            wt = p3w.tile([128, 4, 128], BF16, tag="wt")
            nc.sync.dma_start(wt[:, :, :], wh[:, :, hc * 128:(hc + 1) * 128])
            hpad = p3h.tile([128, 2 * PAD + N], BF16, tag="hpad")
            nc.vector.memset(hpad[:, 0:PAD], 0.0)
            nc.vector.memset(hpad[:, PAD + N:], 0.0)
            for cp2 in range(4):
                c0 = 2 * cp2
                ps = p3p.tile([128, 1024], F32, tag="hps")
                for kt in range(4):
                    nc.tensor.matmul(ps[:, 0:512], wt[:, kt, :],
                                     zT[:, kt, c0 * 512:(c0 + 1) * 512],
                                     start=(kt == 0), stop=(kt == 3))
                    nc.tensor.matmul(ps[:, 512:1024], wt[:, kt, :],
                                     zT[:, kt, (c0 + 1) * 512:(c0 + 2) * 512],
                                     start=(kt == 0), stop=(kt == 3))
                nc.scalar.activation(hpad[:, PAD + c0 * 512:PAD + (c0 + 2) * 512], ps[:, :],
                                     AF.Silu, bias=bh_sb[:, hc:hc + 1], scale=1.0)
            return hpad

        def convpost(hc, hpad):
            if hc % 4 == 0:
                s4_new = p3q.tile([128, NT, 4, 128], BF16, tag="strips4")
                state["strips4"] = s4_new
            strips4 = state["strips4"]
            acc = p3.tile([128, N], BF16, tag="acc")
            if hc in CONV_PE_HID:
                hp8 = _build_hp8(nc, p3, hpad)
                dg = _build_dg(nc, p3, diag_mask, dwh_sb, hc)
                _conv_dr(nc, p3c, hpad, hp8, dg, diag_mask, acc, nc.scalar)
            else:
                hpad1 = p3q.tile([128, 2 * PAD + N], BF16, tag="hpad1")
                nc.scalar.activation(hpad1[:, 0:2 * PAD + N - 2], hpad[:, 1:2 * PAD + N - 1],
                                     AF.Copy)
                scr = p3q.tile([128, N], BF16, tag="convscr")
                _conv_dve(nc, scr, hpad, hpad1, dwh_sb, hc, acc)
            nc.sync.dma_start_transpose(strips4[:, :, hc % 4, :], acc[:, :])
            if hc % 4 == 3:
                q = hc // 4
                nc.sync.dma_start(spill_v[:, :, q, :], strips4[:, :, :, :])
                dst = linkv_sb if hc < 8 else linku_sb
                col = (q % 2) * 512
                lps = p3lin.tile([128, 512], F32, tag="linps")
                for tt in range(NT):
                    nc.tensor.matmul(
                        lps[:, :], lk_str[:, tt, :],
                        strips4[:, tt, :, :].rearrange("p a c -> p (a c)"),
                        start=(tt == 0), stop=(tt == NT - 1))
                nc.scalar.activation(dst[:, col:col + 512], lps[:, :], AF.Copy)

        pending = []
        for hc in range(16):
            pending.append((hc, produce(hc)))
            if len(pending) > 2:
                convpost(*pending.pop(0))
        for item in pending:
            convpost(*item)
    p03.release()

    # P4: attention + gating + LN_o ; z_o kept SBUF-resident (transposed)
    pz = tc.alloc_tile_pool(name="pz", bufs=1)
    z_oT = pz.tile([128, 8, N], BF16)
    with tc.tile_pool(name="p4", bufs=2) as p4, \
         tc.tile_pool(name="p4g", bufs=2) as p4g, \
         tc.tile_pool(name="p4s", bufs=8) as p4s, \
         tc.tile_pool(name="p4p", bufs=2, space="PSUM") as p4p:
        for gpair in range(NG // 2):
            # batch = 2 groups x 2 it-tiles = 4 token tiles; deferred sqrt
            gos = []
            mv4 = p4s.tile([128, 4, 2], F32, tag="mv4")
            vgs, ugs = {}, {}
            for half_g in range(2):
                g = gpair * 2 + half_g
                vg, ug = [], []
                for jh in range(2):
                    vt = p4.tile([128, E2], BF16, tag=f"vg{half_g}{jh}")
                    nc.sync.dma_start(vt[:, :], spill[g * G + jh * 128: g * G + jh * 128 + 128, 0:E2])
                    ut = p4.tile([128, E2], BF16, tag=f"ug{half_g}{jh}")
                    nc.sync.dma_start(ut[:, :], spill[g * G + jh * 128: g * G + jh * 128 + 128, E2:H])
                    vg.append(vt)
                    ug.append(ut)
                vgs[g], ugs[g] = vg, ug
            for j in range(4):
                g = gpair * 2 + j // 2
                it = j % 2
                vg, ug = vgs[g], ugs[g]
                ap_ = p4p.tile([128, 2 * E2], F32, tag="attps")
                islice = slice(g * G + it * 128, g * G + it * 128 + 128)
                for half, (grp, lin) in enumerate(((vg, linkv_sb), (ug, linku_sb))):
                    base = half * E2
                    for e in range(2):
                        for jh in range(2):
                            nc.tensor.matmul(ap_[:, base + e * 512:base + (e + 1) * 512],
                                             attnT[:, g, jh, it * 128:it * 128 + 128],
                                             grp[jh][:, e * 512:(e + 1) * 512],
                                             start=(jh == 0), stop=False)
                        nc.tensor.matmul(ap_[:, base + e * 512:base + (e + 1) * 512],
                                         lq_sb[:, islice], lin[:, e * 512:(e + 1) * 512],
                                         start=False, stop=True)
                avau = p4.tile([128, 2 * E2], BF16, tag="avau")
                nc.scalar.activation(avau[:, :], ap_[:, :], AF.Copy)
                t1 = p4.tile([128, E2], BF16, tag="t1")
                nc.vector.tensor_mul(t1[:, :], ug[it][:, :], avau[:, 0:E2])
                sg = p4.tile([128, E2], BF16, tag="sg")
                nc.scalar.activation(sg[:, :], t1[:, :], AF.Sigmoid)
                t2 = p4.tile([128, E2], BF16, tag="t2")
                nc.gpsimd.tensor_mul(t2[:, :], vg[it][:, :], avau[:, E2:2 * E2])
                go = p4g.tile([128, E2], BF16, tag=f"go{j}")
                sumg = p4s.tile([128, 1], F32, tag="sumg")
                nc.vector.scalar_tensor_tensor(out=go[:, :], in0=t2[:, :], scalar=1.0,
                                               in1=sg[:, :], op0=OP.mult, op1=OP.mult,
                                               accum_out=sumg[:, :])
                g2 = p4.tile([128, E2], BF16, tag="g2")
                sumg2 = p4s.tile([128, 1], F32, tag="sumg2")
                nc.scalar.activation(g2[:, :], go[:, :], AF.Square, accum_out=sumg2[:, :])
                nc.vector.tensor_scalar_mul(mv4[:, j, 0:1], sumg[:, :], 1.0 / E2)
                mm = p4s.tile([128, 1], F32, tag="mm")
                nc.vector.tensor_scalar(out=mm[:, :], in0=mv4[:, j, 0:1], scalar1=mv4[:, j, 0:1],
                                        scalar2=-1.0, op0=OP.mult, op1=OP.mult)
                nc.vector.tensor_scalar(out=mv4[:, j, 1:2], in0=sumg2[:, :], scalar1=1.0 / E2,
                                        scalar2=mm[:, :], op0=OP.mult, op1=OP.add)
                gos.append(go)
            # batched rsqrt over the 4 tiles (one Sqrt act -> one table load)
            rstd4 = p4s.tile([128, 4], F32, tag="rstd4")
            nc.scalar.activation(rstd4[:, :], mv4[:, :, 1], AF.Sqrt, bias=eps_sb[:, :], scale=1.0)
            nc.vector.reciprocal(rstd4[:, :], rstd4[:, :])
            for j in range(4):
                g = gpair * 2 + j // 2
                it = j % 2
                nmu = p4s.tile([128, 1], F32, tag="nmu4")
                nc.vector.tensor_scalar(out=nmu[:, :], in0=mv4[:, j, 0:1],
                                        scalar1=rstd4[:, j:j + 1],
                                        scalar2=-1.0, op0=OP.mult, op1=OP.mult)
                zo = p4.tile([128, E2], BF16, tag="zo")
                nc.vector.tensor_scalar(out=zo[:, :], in0=gos[j][:, :], scalar1=rstd4[:, j:j + 1],
                                        scalar2=nmu[:, :], op0=OP.mult, op1=OP.add)
                tti = g * 2 + it
                nc.sync.dma_start_transpose(z_oT[:, :, tti * 128:(tti + 1) * 128], zo[:, :])
    qs.release()

    # P5: output FFConvM reading z_oT from SBUF
    pvo = tc.alloc_tile_pool(name="pvo", bufs=1, side="right")
    vo_big = pvo.tile([128, NT, 4, 128], BF16)
    with tc.tile_pool(name="p5", bufs=2) as p5, \
         tc.tile_pool(name="p5q", bufs=1) as p5q, \
         tc.tile_pool(name="p5p", bufs=2, space="PSUM") as p5p, \
         tc.tile_pool(name="p5c", bufs=2, space="PSUM") as p5c:

        def produce5(oc):
            hpad = p5.tile([128, 2 * PAD + N], BF16, tag="hpad5")
            nc.vector.memset(hpad[:, 0:PAD], 0.0)
            nc.vector.memset(hpad[:, PAD + N:], 0.0)
            for cp2 in range(4):
                c0 = 2 * cp2
                ps = p5p.tile([128, 1024], F32, tag="ops")
                for kt in range(8):
                    nc.tensor.matmul(ps[:, 0:512], wo_sb[:, kt, oc * 128:(oc + 1) * 128],
                                     z_oT[:, kt, c0 * 512:(c0 + 1) * 512],
                                     start=(kt == 0), stop=(kt == 7))
                    nc.tensor.matmul(ps[:, 512:1024], wo_sb[:, kt, oc * 128:(oc + 1) * 128],
                                     z_oT[:, kt, (c0 + 1) * 512:(c0 + 2) * 512],
                                     start=(kt == 0), stop=(kt == 7))
                nc.scalar.activation(hpad[:, PAD + c0 * 512:PAD + (c0 + 2) * 512], ps[:, :],
                                     AF.Silu, bias=bo_sb[:, oc:oc + 1], scale=1.0)
            return hpad

        def convpost5(oc, hpad):
            acc = p5.tile([128, N], BF16, tag="acc5")
            if oc in CONV_PE_O:
                hp8 = _build_hp8(nc, p5, hpad)
                dg = _build_dg(nc, p5, diag_mask, dwo_sb, oc)
                _conv_dr(nc, p5c, hpad, hp8, dg, diag_mask, acc, nc.scalar)
            else:
                hpad1 = p5q.tile([128, 2 * PAD + N], BF16, tag="hpad51")
                nc.scalar.activation(hpad1[:, 0:2 * PAD + N - 2], hpad[:, 1:2 * PAD + N - 1],
                                     AF.Copy)
                scr = p5q.tile([128, N], BF16, tag="convscr5")
                _conv_dve(nc, scr, hpad, hpad1, dwo_sb, oc, acc)
            nc.sync.dma_start_transpose(vo_big[:, :, oc, :], acc[:, :])

        prev = None
        for oc in range(4):
            hp = produce5(oc)
            if prev is not None:
                convpost5(*prev)
            prev = (oc, hp)
        convpost5(*prev)
    pz.release()

    # P6: residual, 4 token-tiles per DMA; x prefetched during P5, adds split
    # across the vector and gpsimd engines to shorten the tail.
    with tc.tile_pool(name="p6x", bufs=1) as p6x, \
         tc.tile_pool(name="p6", bufs=4) as p6:
        xts = []
        for c in range(NT // 4):
            t0 = c * 512
            xt = p6x.tile([128, 4, D], F32, tag=f"xt6{c}")
            nc.sync.dma_start(xt[:, :, :],
                              x[t0:t0 + 512, :].rearrange("(q p) d -> p q d", p=128))
            xts.append(xt)
        for c in range(NT // 4):
            t0 = c * 512
            of = p6.tile([128, 4, D], F32, tag="of")
            eng = nc.vector if c % 2 == 0 else nc.gpsimd
            eng.tensor_add(
                of[:, :, :].rearrange("p q d -> p (q d)"),
                xts[c][:, :, :].rearrange("p q d -> p (q d)"),
                vo_big[:, 4 * c:4 * c + 4, :, :].rearrange("p q a c -> p (q a c)"))
            nc.sync.dma_start(out[t0:t0 + 512, :].rearrange("(q p) d -> p q d", p=128),
                              of[:, :, :])
    pvo.release()
    consts.release()


def _build_nc():
    nc = bacc.Bacc("TRN2", target_bir_lowering=False, debug=False)
    x = nc.dram_tensor("x", [N, D], F32, kind="ExternalInput")
    wh = nc.dram_tensor("wh", [128, 4, H], BF16, kind="ExternalInput")
    wqk = nc.dram_tensor("wqk", [128, 4, QK], BF16, kind="ExternalInput")
    wo = nc.dram_tensor("wo", [128, 8, D], BF16, kind="ExternalInput")
    bh = nc.dram_tensor("bh", [128, 16], F32, kind="ExternalInput")
    bqk = nc.dram_tensor("bqk", [128, 1], F32, kind="ExternalInput")
    bo = nc.dram_tensor("bo", [128, 4], F32, kind="ExternalInput")
    dwh = nc.dram_tensor("dwh", [128, 16, KTAPS], F32, kind="ExternalInput")
    dwqk = nc.dram_tensor("dwqk", [128, 1, KTAPS], F32, kind="ExternalInput")
    dwo = nc.dram_tensor("dwo", [128, 4, KTAPS], F32, kind="ExternalInput")
    gb = nc.dram_tensor("gb", [128, 8], F32, kind="ExternalInput")
    out = nc.dram_tensor("out", [N, D], F32, kind="ExternalOutput")
    spill = nc.dram_tensor("spill", [N, H], BF16)
    with tile.TileContext(nc) as tc:
        _emit(nc, tc, x, wh, wqk, wo, bh, bqk, bo, dwh, dwqk, dwo, gb, out, spill)
    nc.compile()
    return nc


def prep_inputs(inputs):
    f32 = np.float32
    bf = ml_dtypes.bfloat16
    W_h = np.asarray(inputs["W_h"], f32)
    W_qk = np.asarray(inputs["W_qk"], f32)
    W_o = np.asarray(inputs["W_o"], f32)
    whp = np.asarray(inputs["ln_h_g"], f32)[:, None] * W_h
    bhp = np.asarray(inputs["ln_h_b"], f32) @ W_h + np.asarray(inputs["b_h"], f32)
    wqkp = np.asarray(inputs["ln_qk_g"], f32)[:, None] * W_qk
    bqkp = np.asarray(inputs["ln_qk_b"], f32) @ W_qk + np.asarray(inputs["b_qk"], f32)
    wop = np.asarray(inputs["ln_o_g"], f32)[:, None] * W_o
    bop = np.asarray(inputs["ln_o_b"], f32) @ W_o + np.asarray(inputs["b_o"], f32)
    gamma = np.asarray(inputs["gamma"], f32).copy()
    beta = np.asarray(inputs["beta"], f32).copy()
    gamma[0] /= G
    beta[0] /= G
    gamma[3] /= N
    beta[3] /= N

    def lhsT(w, ktiles):
        return np.ascontiguousarray(w.reshape(ktiles, 128, -1).transpose(1, 0, 2)).astype(bf)

    def chan(v, ntiles):
        return np.ascontiguousarray(v.reshape(ntiles, 128).T).astype(f32)

    def dwl(dw, ntiles):
        return np.ascontiguousarray(
            dw.T.reshape(ntiles, 128, KTAPS).transpose(1, 0, 2)).astype(f32)

    return {
        "wh": lhsT(whp, 4), "wqk": lhsT(wqkp, 4), "wo": lhsT(wop, 8),
        "bh": chan(bhp, 16), "bqk": chan(bqkp, 1), "bo": chan(bop, 4),
        "dwh": dwl(np.asarray(inputs["dw_h"], f32), 16),
        "dwqk": dwl(np.asarray(inputs["dw_qk"], f32), 1),
        "dwo": dwl(np.asarray(inputs["dw_o"], f32), 4),
        "gb": np.concatenate([gamma.T, beta.T], axis=1).astype(f32),
    }


_NC = None


def get_nc():
    global _NC
    if _NC is None:
        _NC = _build_nc()
    return _NC


def make_in_maps(inputs):
    x = np.asarray(inputs["x"], np.float32)
    B = x.shape[0]
    prep = prep_inputs(inputs)
    return [{"x": np.ascontiguousarray(x[b]), **prep} for b in range(B)]


def kernel(**inputs):
    nc = get_nc()
    in_maps = make_in_maps(inputs)
    res = bass_utils.run_bass_kernel_spmd(nc, in_maps, core_ids=list(range(8)))
    out = np.stack([res.results[b]["out"] for b in range(8)], axis=0)
    return out.astype(np.float32)


# revision 34
# speedup vs baseline: 1.1231x; 1.0027x over previous
"""Self-contained TRN2 kernel for nn_FLASH_ShareA_FFConvM_FlashAttn.

kernel(**inputs) takes the full (unsharded) inputs from setup_inputs() and
returns the full (B, N, D) float32 output. Internally: data-parallel over the
batch — one batch sample per NeuronCore, 8 cores, no collectives.

v2: depthwise convs via fp8 DoubleRow diag matmuls on the PE; z_o kept
SBUF-resident (no DRAM roundtrip); batched Sqrt in the gating phase to avoid
activation-table thrash; shift copies on the scalar engine.
"""
import sys

if "/opt/trn_rl_repo" not in sys.path:
    sys.path.insert(0, "/opt/trn_rl_repo")

import numpy as np
import ml_dtypes
import concourse.bass as bass
import concourse.bacc as bacc
import concourse.mybir as mybir
import concourse.tile as tile
from concourse import bass_utils

F32 = mybir.dt.float32
BF16 = mybir.dt.bfloat16
FP8 = mybir.dt.float8e4
AF = mybir.ActivationFunctionType
OP = mybir.AluOpType
PM = mybir.MatmulPerfMode

N, D, H, QK, G = 4096, 512, 2048, 128, 256
NG = N // G
NT = N // 128
KTAPS = 17
PAD = 8
E2 = 2 * D
EPS = 1e-5
W8 = 2 * PAD + N + 16  # fp8 double-plane buffer width (4128, mult of 16)

# channel tiles of the depthwise convs on the PE (fp8 DoubleRow diag matmuls);
# the rest run on the vector engine. DVE tiles are scheduled early so no
# phase ends on a long vector-engine tail.
CONV_PE_HID = frozenset(range(16)) - {5, 9}
CONV_PE_QK = True
CONV_PE_O = frozenset({0, 2, 3})
# DoubleRow tap pairs (k, k+2) matching a +2-column plane-1 shift
PAIR_KS = [0, 1, 4, 5, 8, 9, 12, 13]


def _conv_dve(nc, scratch, hpad, hpad1, dw_sb, dwi, acc):
    """acc = h + conv(h) via tensor_scalar products (4x) + tensor_tensor adds (2x)."""
    for k in range(KTAPS):
        s = k - PAD
        if s % 2 == 0:
            src, off = hpad, PAD + s
        else:
            src, off = hpad1, PAD - 1 + s
        if k == 0:
            nc.vector.scalar_tensor_tensor(
                out=acc[:, :], in0=src[:, off:off + N], scalar=dw_sb[:, dwi, 0:1],
                in1=hpad[:, PAD:PAD + N], op0=OP.mult, op1=OP.add)
        else:
            nc.vector.tensor_scalar(out=scratch[:, :], in0=src[:, off:off + N],
                                    scalar1=dw_sb[:, dwi, k:k + 1], scalar2=None,
                                    op0=OP.mult)
            nc.vector.tensor_add(acc[:, :], acc[:, :], scratch[:, :])


def _build_hp8(nc, pool, hpad):
    """fp8 double-plane buffer: plane0 = fp8(hpad), plane1 = plane0 shifted +2."""
    hp8 = pool.tile([128, 2, W8], FP8, tag="hp8")
    nc.scalar.activation(hp8[:, 0, 0:2 * PAD + N], hpad[:, :], AF.Copy)
    nc.vector.memset(hp8[:, 0, 2 * PAD + N:W8], 0.0)
    nc.sync.dma_start(hp8[:, 1, 0:W8 - 2], hp8[:, 0, 2:W8])
    nc.vector.memset(hp8[:, 1, W8 - 2:W8], 0.0)
    return hp8


def _build_dg(nc, pool, diag_mask, dw_sb, dwi):
    """fp8 diag weight pairs [128, 9, 2, 128] for DoubleRow conv."""
    dg = pool.tile([128, 9, 2, 128], FP8, tag="dg")
    for j, k in enumerate(PAIR_KS):
        nc.vector.tensor_scalar(out=dg[:, j, 0, :], in0=diag_mask[:, :],
                                scalar1=dw_sb[:, dwi, k:k + 1], scalar2=None, op0=OP.mult)
        nc.vector.tensor_scalar(out=dg[:, j, 1, :], in0=diag_mask[:, :],
                                scalar1=dw_sb[:, dwi, k + 2:k + 3], scalar2=None, op0=OP.mult)
    nc.vector.tensor_scalar(out=dg[:, 8, 0, :], in0=diag_mask[:, :],
                            scalar1=dw_sb[:, dwi, 16:17], scalar2=None, op0=OP.mult)
    nc.vector.memset(dg[:, 8, 1, :], 0.0)
    return dg


def _conv_dr(nc, psum_pool, hpad, hp8, dg, diag_mask, acc, evac):
    """acc = h + conv(h): bf16 identity + 9 fp8 DoubleRow diag pair-matmuls."""
    for half in range(4):
        ps0 = psum_pool.tile([128, 512], F32, tag="convdr")
        ps1 = psum_pool.tile([128, 512], F32, tag="convdr")
        ps = [ps0, ps1]
        bases = [PAD + (2 * half + i) * 512 for i in range(2)]
        for i in range(2):
            nc.tensor.matmul(ps[i][:, :], diag_mask[:, :], hpad[:, bases[i]:bases[i] + 512],
                             start=True, stop=False, skip_group_check=True)
        for j, k in enumerate(PAIR_KS + [16]):
            s = k - PAD
            last = j == 8
            for i in range(2):
                nc.tensor.matmul(ps[i][:, :], dg[:, j, :, :],
                                 hp8[:, :, bases[i] + s:bases[i] + s + 512],
                                 start=False, stop=last, perf_mode=PM.DoubleRow,
                                 skip_group_check=True)
        for i in range(2):
            c = 2 * half + i
            evac.activation(acc[:, c * 512:(c + 1) * 512], ps[i][:, :], AF.Copy)


def _emit(nc, tc, x, wh, wqk, wo, bh, bqk, bo, dwh, dwqk, dwo, gb, out, spill):
    consts = tc.alloc_tile_pool(name="consts", bufs=1)
    wqk_sb = consts.tile([128, 4, QK], BF16)
    nc.sync.dma_start(wqk_sb[:, :, :], wqk.ap())
    wo_sb = consts.tile([128, 8, D], BF16)
    nc.sync.dma_start(wo_sb[:, :, :], wo.ap())
    bh_sb = consts.tile([128, 16], F32)
    nc.sync.dma_start(bh_sb[:, :], bh.ap())
    bqk_sb = consts.tile([128, 1], F32)
    nc.sync.dma_start(bqk_sb[:, :], bqk.ap())
    bo_sb = consts.tile([128, 4], F32)
    nc.sync.dma_start(bo_sb[:, :], bo.ap())
    dwh_sb = consts.tile([128, 16, KTAPS], F32)
    nc.sync.dma_start(dwh_sb[:, :, :], dwh.ap())
    dwqk_sb = consts.tile([128, 1, KTAPS], F32)
    nc.sync.dma_start(dwqk_sb[:, :, :], dwqk.ap())
    dwo_sb = consts.tile([128, 4, KTAPS], F32)
    nc.sync.dma_start(dwo_sb[:, :, :], dwo.ap())
    gb_sb = consts.tile([128, 8], F32)
    nc.sync.dma_start(gb_sb[:, :], gb.ap())
    eps_sb = consts.tile([128, 1], F32)
    nc.vector.memset(eps_sb[:, :], EPS)

    iota_row = consts.tile([128, 128], F32)
    nc.gpsimd.iota(iota_row[:, :], pattern=[[1, 128]], base=0, channel_multiplier=0,
                   allow_small_or_imprecise_dtypes=True)
    iota_p = consts.tile([128, 1], F32)
    nc.gpsimd.iota(iota_p[:, :], pattern=[[0, 1]], base=0, channel_multiplier=1,
                   allow_small_or_imprecise_dtypes=True)
    diag_mask = consts.tile([128, 128], BF16)
    nc.vector.tensor_scalar(out=diag_mask[:, :], in0=iota_row[:, :],
                            scalar1=iota_p[:, :], scalar2=None, op0=OP.is_equal)

    p03 = tc.alloc_tile_pool(name="p03", bufs=1)     # P0-P3: zT
    zT = p03.tile([128, 4, N], BF16)
    qs = tc.alloc_tile_pool(name="qside", bufs=1, side="right")    # P1-P4
    attnT = qs.tile([128, NG, 2, G], BF16)
    lq_sb = qs.tile([128, N], BF16)
    lk_str = qs.tile([128, NT, 128], BF16)
    linkv_sb = qs.tile([128, E2], BF16)
    linku_sb = qs.tile([128, E2], BF16)

    # P0 + P1 head: token-shifted LayerNorm, with the qk GEMM interleaved
    # per 512-token chunk (keeps the PE warm during the load/normalize phase).
    with tc.tile_pool(name="p0", bufs=3) as p0, \
         tc.tile_pool(name="p0s", bufs=8) as p0s, \
         tc.tile_pool(name="p1", bufs=1) as p1, \
         tc.tile_pool(name="p1p", bufs=2, space="PSUM") as p1p:
        qkp = p1.tile([128, 2 * PAD + N], BF16, tag="qkpad")
        nc.vector.memset(qkp[:, 0:PAD], 0.0)
        nc.vector.memset(qkp[:, PAD + N:], 0.0)
        for ch in range(8):
            t0 = ch * 512
            xt4 = p0.tile([128, 4, D], F32, tag="xt4")
            if ch == 0:
                nc.vector.memset(xt4[0:1, 0, 0:D // 2], 0.0)
                nc.sync.dma_start(xt4[1:128, 0, 0:D // 2], x[0:127, 0:D // 2])
                nc.sync.dma_start(
                    xt4[:, 1:4, 0:D // 2],
                    x[127:511, 0:D // 2].rearrange("(q p) d -> p q d", p=128))
            else:
                nc.sync.dma_start(
                    xt4[:, :, 0:D // 2],
                    x[t0 - 1:t0 + 511, 0:D // 2].rearrange("(q p) d -> p q d", p=128))
            nc.sync.dma_start(
                xt4[:, :, D // 2:D],
                x[t0:t0 + 512, D // 2:D].rearrange("(q p) d -> p q d", p=128))
            mv4 = p0s.tile([128, 4, 2], F32, tag="mv4p0")
            for q in range(4):
                st6 = p0s.tile([128, 6], F32, tag="st6")
                nc.vector.bn_stats(st6[:, :], xt4[:, q, :])
                nc.vector.bn_aggr(mv4[:, q, :], st6[:, :])
            rstd4 = p0s.tile([128, 4], F32, tag="rstd4p0")
            nc.scalar.activation(rstd4[:, :], mv4[:, :, 1], AF.Sqrt, bias=eps_sb[:, :],
                                 scale=1.0)
            nc.vector.reciprocal(rstd4[:, :], rstd4[:, :])
            for q in range(4):
                nmu = p0s.tile([128, 1], F32, tag="nmu")
                nc.vector.tensor_scalar(out=nmu[:, :], in0=mv4[:, q, 0:1],
                                        scalar1=rstd4[:, q:q + 1],
                                        scalar2=-1.0, op0=OP.mult, op1=OP.mult)
                zt = p0.tile([128, D], BF16, tag="zt")
                nc.vector.tensor_scalar(out=zt[:, :], in0=xt4[:, q, :],
                                        scalar1=rstd4[:, q:q + 1],
                                        scalar2=nmu[:, :], op0=OP.mult, op1=OP.add)
                nc.sync.dma_start_transpose(zT[:, :, t0 + q * 128:t0 + (q + 1) * 128],
                                            zt[:, :])
            ps = p1p.tile([128, 512], F32, tag="qkps")
            for kt in range(4):
                nc.tensor.matmul(ps[:, :], wqk_sb[:, kt, :], zT[:, kt, ch * 512:(ch + 1) * 512],
                                 start=(kt == 0), stop=(kt == 3))
            nc.scalar.activation(qkp[:, PAD + ch * 512:PAD + (ch + 1) * 512], ps[:, :],
                                 AF.Silu, bias=bqk_sb[:, :], scale=1.0)

        qkc = p1.tile([128, N], BF16, tag="qkc")
        if CONV_PE_QK:
            with tc.tile_pool(name="p1cp", bufs=2, space="PSUM") as p1cp:
                hp8 = _build_hp8(nc, p1, qkp)
                dg = _build_dg(nc, p1, diag_mask, dwqk_sb, 0)
                _conv_dr(nc, p1cp, qkp, hp8, dg, diag_mask, qkc, nc.scalar)
        else:
            qkp1 = p1.tile([128, 2 * PAD + N], BF16, tag="qkpad1")
            nc.scalar.activation(qkp1[:, 0:2 * PAD + N - 2], qkp[:, 1:2 * PAD + N - 1], AF.Copy)
            qscr = p1.tile([128, N], BF16, tag="qscr")
            _conv_dve(nc, qscr, qkp, qkp1, dwqk_sb, 0, qkc)
        qq = p1.tile([128, N], BF16, tag="qq")
        qkk = p1.tile([128, N], BF16, tag="qkk")
        lkk = p1.tile([128, N], BF16, tag="lkk")
        # sim's operands (qq, qkk) first so the attention-weight pipeline can
        # start; lq/lkk (only needed from P3's first lin matmul on) after.
        for i, dst in ((0, qq), (2, qkk)):
            nc.vector.tensor_scalar(out=dst[:, :], in0=qkc[:, :], scalar1=gb_sb[:, i:i + 1],
                                    scalar2=gb_sb[:, 4 + i:5 + i], op0=OP.mult, op1=OP.add)
        with tc.tile_pool(name="p1sim", bufs=4, space="PSUM") as p1sim:
            for g in range(NG):
                for jh in range(2):
                    sp = p1sim.tile([128, G], F32, tag="simps")
                    nc.tensor.matmul(sp[:, :], qkk[:, g * G + jh * 128: g * G + jh * 128 + 128],
                                     qq[:, g * G:(g + 1) * G], start=True, stop=True)
                    rel = p1.tile([128, G], BF16, tag="rel")
                    nc.scalar.activation(rel[:, :], sp[:, :], AF.Relu)
                    nc.vector.tensor_mul(attnT[:, g, jh, :], rel[:, :], rel[:, :])
        for i, dst in ((1, lq_sb), (3, lkk)):
            nc.vector.tensor_scalar(out=dst[:, :], in0=qkc[:, :], scalar1=gb_sb[:, i:i + 1],
                                    scalar2=gb_sb[:, 4 + i:5 + i], op0=OP.mult, op1=OP.add)
        nc.sync.dma_start_transpose(lk_str[:, :, :], lkk[:, :])

    # P3: hidden + conv + spill + lin_kv/lin_ku
    spill_v = spill.ap().rearrange("(tt p) (q c4) -> p tt q c4", p=128, c4=512)
    with tc.tile_pool(name="p3w", bufs=4) as p3w, \
         tc.tile_pool(name="p3h", bufs=3) as p3h, \
         tc.tile_pool(name="p3", bufs=2) as p3, \
         tc.tile_pool(name="p3q", bufs=1) as p3q, \
         tc.tile_pool(name="p3p", bufs=2, space="PSUM") as p3p, \
         tc.tile_pool(name="p3c", bufs=2, space="PSUM") as p3c, \
         tc.tile_pool(name="p3lin", bufs=2, space="PSUM") as p3lin:
        state = {"strips4": None}

        def produce(hc):
            wt = p3w.tile([128, 4, 128], BF16, tag="wt")
            nc.sync.dma_start(wt[:, :, :], wh[:, :, hc * 128:(hc + 1) * 128])
            hpad = p3h.tile([128, 2 * PAD + N], BF16, tag="hpad")
            nc.vector.memset(hpad[:, 0:PAD], 0.0)
            nc.vector.memset(hpad[:, PAD + N:], 0.0)
            for cp2 in range(4):
                c0 = 2 * cp2
                ps = p3p.tile([128, 1024], F32, tag="hps")
                for kt in range(4):
                    nc.tensor.matmul(ps[:, 0:512], wt[:, kt, :],
                                     zT[:, kt, c0 * 512:(c0 + 1) * 512],
                                     start=(kt == 0), stop=(kt == 3))
                    nc.tensor.matmul(ps[:, 512:1024], wt[:, kt, :],
                                     zT[:, kt, (c0 + 1) * 512:(c0 + 2) * 512],
                                     start=(kt == 0), stop=(kt == 3))
                nc.scalar.activation(hpad[:, PAD + c0 * 512:PAD + (c0 + 2) * 512], ps[:, :],
                                     AF.Silu, bias=bh_sb[:, hc:hc + 1], scale=1.0)
            return hpad

        def convpost(hc, hpad):
            if hc % 4 == 0:
                s4_new = p3q.tile([128, NT, 4, 128], BF16, tag="strips4")
                state["strips4"] = s4_new
            strips4 = state["strips4"]
            acc = p3.tile([128, N], BF16, tag="acc")
            if hc in CONV_PE_HID:
                hp8 = _build_hp8(nc, p3, hpad)
                dg = _build_dg(nc, p3, diag_mask, dwh_sb, hc)
                _conv_dr(nc, p3c, hpad, hp8, dg, diag_mask, acc, nc.scalar)
            else:
                hpad1 = p3q.tile([128, 2 * PAD + N], BF16, tag="hpad1")
                nc.scalar.activation(hpad1[:, 0:2 * PAD + N - 2], hpad[:, 1:2 * PAD + N - 1],
                                     AF.Copy)
                scr = p3q.tile([128, N], BF16, tag="convscr")
                _conv_dve(nc, scr, hpad, hpad1, dwh_sb, hc, acc)
            nc.sync.dma_start_transpose(strips4[:, :, hc % 4, :], acc[:, :])
            if hc % 4 == 3:
                q = hc // 4
                nc.sync.dma_start(spill_v[:, :, q, :], strips4[:, :, :, :])
                dst = linkv_sb if hc < 8 else linku_sb
                col = (q % 2) * 512
                lps = p3lin.tile([128, 512], F32, tag="linps")
                for tt in range(NT):
                    nc.tensor.matmul(
                        lps[:, :], lk_str[:, tt, :],
                        strips4[:, tt, :, :].rearrange("p a c -> p (a c)"),
                        start=(tt == 0), stop=(tt == NT - 1))
                nc.scalar.activation(dst[:, col:col + 512], lps[:, :], AF.Copy)

        pending = []
        for hc in range(16):
            pending.append((hc, produce(hc)))
            if len(pending) > 2:
                convpost(*pending.pop(0))
        for item in pending:
            convpost(*item)
    p03.release()

    # P4: attention + gating + LN_o ; z_o kept SBUF-resident (transposed)
    pz = tc.alloc_tile_pool(name="pz", bufs=1)
    z_oT = pz.tile([128, 8, N], BF16)
    with tc.tile_pool(name="p4", bufs=2) as p4, \
         tc.tile_pool(name="p4g", bufs=2) as p4g, \
         tc.tile_pool(name="p4s", bufs=8) as p4s, \
         tc.tile_pool(name="p4p", bufs=2, space="PSUM") as p4p:
        for gpair in range(NG // 2):
            # batch = 2 groups x 2 it-tiles = 4 token tiles; deferred sqrt
            gos = []
            mv4 = p4s.tile([128, 4, 2], F32, tag="mv4")
            vgs, ugs = {}, {}
            for half_g in range(2):
                g = gpair * 2 + half_g
                vt = p4.tile([128, 2, E2], BF16, tag=f"vg{half_g}")
                nc.sync.dma_start(
                    vt[:, :, :],
                    spill[g * G:(g + 1) * G, 0:E2].rearrange("(jh p) c -> p jh c", p=128))
                ut = p4.tile([128, 2, E2], BF16, tag=f"ug{half_g}")
                nc.sync.dma_start(
                    ut[:, :, :],
                    spill[g * G:(g + 1) * G, E2:H].rearrange("(jh p) c -> p jh c", p=128))
                vgs[g], ugs[g] = vt, ut
            for j in range(4):
                g = gpair * 2 + j // 2
                it = j % 2
                vg, ug = vgs[g], ugs[g]
                ap_ = p4p.tile([128, 2 * E2], F32, tag="attps")
                islice = slice(g * G + it * 128, g * G + it * 128 + 128)
                for half, (grp, lin) in enumerate(((vg, linkv_sb), (ug, linku_sb))):
                    base = half * E2
                    for e in range(2):
                        for jh in range(2):
                            nc.tensor.matmul(ap_[:, base + e * 512:base + (e + 1) * 512],
                                             attnT[:, g, jh, it * 128:it * 128 + 128],
                                             grp[:, jh, e * 512:(e + 1) * 512],
                                             start=(jh == 0), stop=False)
                        nc.tensor.matmul(ap_[:, base + e * 512:base + (e + 1) * 512],
                                         lq_sb[:, islice], lin[:, e * 512:(e + 1) * 512],
                                         start=False, stop=True)
                avau = p4.tile([128, 2 * E2], BF16, tag="avau")
                nc.scalar.activation(avau[:, :], ap_[:, :], AF.Copy)
                t1 = p4.tile([128, E2], BF16, tag="t1")
                nc.vector.tensor_mul(t1[:, :], ug[:, it, :], avau[:, 0:E2])
                sg = p4.tile([128, E2], BF16, tag="sg")
                nc.scalar.activation(sg[:, :], t1[:, :], AF.Sigmoid)
                t2 = p4.tile([128, E2], BF16, tag="t2")
                nc.gpsimd.tensor_mul(t2[:, :], vg[:, it, :], avau[:, E2:2 * E2])
                go = p4g.tile([128, E2], BF16, tag=f"go{j}")
                sumg = p4s.tile([128, 1], F32, tag="sumg")
                nc.vector.scalar_tensor_tensor(out=go[:, :], in0=t2[:, :], scalar=1.0,
                                               in1=sg[:, :], op0=OP.mult, op1=OP.mult,
                                               accum_out=sumg[:, :])
                g2 = p4.tile([128, E2], BF16, tag="g2")
                sumg2 = p4s.tile([128, 1], F32, tag="sumg2")
                nc.scalar.activation(g2[:, :], go[:, :], AF.Square, accum_out=sumg2[:, :])
                nc.vector.tensor_scalar_mul(mv4[:, j, 0:1], sumg[:, :], 1.0 / E2)
                mm = p4s.tile([128, 1], F32, tag="mm")
                nc.vector.tensor_scalar(out=mm[:, :], in0=mv4[:, j, 0:1], scalar1=mv4[:, j, 0:1],
                                        scalar2=-1.0, op0=OP.mult, op1=OP.mult)
                nc.vector.tensor_scalar(out=mv4[:, j, 1:2], in0=sumg2[:, :], scalar1=1.0 / E2,
                                        scalar2=mm[:, :], op0=OP.mult, op1=OP.add)
                gos.append(go)
            # batched rsqrt over the 4 tiles (one Sqrt act -> one table load)
            rstd4 = p4s.tile([128, 4], F32, tag="rstd4")
            nc.scalar.activation(rstd4[:, :], mv4[:, :, 1], AF.Sqrt, bias=eps_sb[:, :], scale=1.0)
            nc.vector.reciprocal(rstd4[:, :], rstd4[:, :])
            for j in range(4):
                g = gpair * 2 + j // 2
                it = j % 2
                nmu = p4s.tile([128, 1], F32, tag="nmu4")
                nc.vector.tensor_scalar(out=nmu[:, :], in0=mv4[:, j, 0:1],
                                        scalar1=rstd4[:, j:j + 1],
                                        scalar2=-1.0, op0=OP.mult, op1=OP.mult)
                zo = p4.tile([128, E2], BF16, tag="zo")
                nc.vector.tensor_scalar(out=zo[:, :], in0=gos[j][:, :], scalar1=rstd4[:, j:j + 1],
                                        scalar2=nmu[:, :], op0=OP.mult, op1=OP.add)
                tti = g * 2 + it
                nc.sync.dma_start_transpose(z_oT[:, :, tti * 128:(tti + 1) * 128], zo[:, :])
    qs.release()

    # P5: output FFConvM reading z_oT from SBUF
    pvo = tc.alloc_tile_pool(name="pvo", bufs=1, side="right")
    vo_big = pvo.tile([128, NT, 4, 128], BF16)
    with tc.tile_pool(name="p5", bufs=2) as p5, \
         tc.tile_pool(name="p5q", bufs=1) as p5q, \
         tc.tile_pool(name="p5p", bufs=2, space="PSUM") as p5p, \
         tc.tile_pool(name="p5c", bufs=2, space="PSUM") as p5c:

        def produce5(oc):
            hpad = p5.tile([128, 2 * PAD + N], BF16, tag="hpad5")
            nc.vector.memset(hpad[:, 0:PAD], 0.0)
            nc.vector.memset(hpad[:, PAD + N:], 0.0)
            for cp2 in range(4):
                c0 = 2 * cp2
                ps = p5p.tile([128, 1024], F32, tag="ops")
                for kt in range(8):
                    nc.tensor.matmul(ps[:, 0:512], wo_sb[:, kt, oc * 128:(oc + 1) * 128],
                                     z_oT[:, kt, c0 * 512:(c0 + 1) * 512],
                                     start=(kt == 0), stop=(kt == 7))
                    nc.tensor.matmul(ps[:, 512:1024], wo_sb[:, kt, oc * 128:(oc + 1) * 128],
                                     z_oT[:, kt, (c0 + 1) * 512:(c0 + 2) * 512],
                                     start=(kt == 0), stop=(kt == 7))
                nc.scalar.activation(hpad[:, PAD + c0 * 512:PAD + (c0 + 2) * 512], ps[:, :],
                                     AF.Silu, bias=bo_sb[:, oc:oc + 1], scale=1.0)
            return hpad

        def convpost5(oc, hpad):
            acc = p5.tile([128, N], BF16, tag="acc5")
            if oc in CONV_PE_O:
                hp8 = _build_hp8(nc, p5, hpad)
                dg = _build_dg(nc, p5, diag_mask, dwo_sb, oc)
                _conv_dr(nc, p5c, hpad, hp8, dg, diag_mask, acc, nc.scalar)
            else:
                hpad1 = p5q.tile([128, 2 * PAD + N], BF16, tag="hpad51")
                nc.scalar.activation(hpad1[:, 0:2 * PAD + N - 2], hpad[:, 1:2 * PAD + N - 1],
                                     AF.Copy)
                scr = p5q.tile([128, N], BF16, tag="convscr5")
                _conv_dve(nc, scr, hpad, hpad1, dwo_sb, oc, acc)
            nc.sync.dma_start_transpose(vo_big[:, :, oc, :], acc[:, :])

        prev = None
        for oc in range(4):
            hp = produce5(oc)
            if prev is not None:
                convpost5(*prev)
            prev = (oc, hp)
        convpost5(*prev)
    pz.release()

    # P6: residual, 4 token-tiles per DMA; x prefetched during P5, adds split
    # across the vector and gpsimd engines to shorten the tail.
    with tc.tile_pool(name="p6x", bufs=1) as p6x, \
         tc.tile_pool(name="p6", bufs=4) as p6:
        xts = []
        for c in range(NT // 4):
            t0 = c * 512
            xt = p6x.tile([128, 4, D], F32, tag=f"xt6{c}")
            nc.sync.dma_start(xt[:, :, :],
                              x[t0:t0 + 512, :].rearrange("(q p) d -> p q d", p=128))
            xts.append(xt)
        for c in range(NT // 4):
            t0 = c * 512
            of = p6.tile([128, 4, D], F32, tag="of")
            eng = nc.vector if c % 2 == 0 else nc.gpsimd
            eng.tensor_add(
                of[:, :, :].rearrange("p q d -> p (q d)"),
                xts[c][:, :, :].rearrange("p q d -> p (q d)"),
                vo_big[:, 4 * c:4 * c + 4, :, :].rearrange("p q a c -> p (q a c)"))
            nc.sync.dma_start(out[t0:t0 + 512, :].rearrange("(q p) d -> p q d", p=128),
                              of[:, :, :])
    pvo.release()
    consts.release()


def _build_nc():
    nc = bacc.Bacc("TRN2", target_bir_lowering=False, debug=False)
    x = nc.dram_tensor("x", [N, D], F32, kind="ExternalInput")
    wh = nc.dram_tensor("wh", [128, 4, H], BF16, kind="ExternalInput")
    wqk = nc.dram_tensor("wqk", [128, 4, QK], BF16, kind="ExternalInput")
    wo = nc.dram_tensor("wo", [128, 8, D], BF16, kind="ExternalInput")
    bh = nc.dram_tensor("bh", [128, 16], F32, kind="ExternalInput")
    bqk = nc.dram_tensor("bqk", [128, 1], F32, kind="ExternalInput")
    bo = nc.dram_tensor("bo", [128, 4], F32, kind="ExternalInput")
    dwh = nc.dram_tensor("dwh", [128, 16, KTAPS], F32, kind="ExternalInput")
    dwqk = nc.dram_tensor("dwqk", [128, 1, KTAPS], F32, kind="ExternalInput")
    dwo = nc.dram_tensor("dwo", [128, 4, KTAPS], F32, kind="ExternalInput")
    gb = nc.dram_tensor("gb", [128, 8], F32, kind="ExternalInput")
    out = nc.dram_tensor("out", [N, D], F32, kind="ExternalOutput")
    spill = nc.dram_tensor("spill", [N, H], BF16)
    with tile.TileContext(nc) as tc:
        _emit(nc, tc, x, wh, wqk, wo, bh, bqk, bo, dwh, dwqk, dwo, gb, out, spill)
    nc.compile()
    return nc


def prep_inputs(inputs):
    f32 = np.float32
    bf = ml_dtypes.bfloat16
    W_h = np.asarray(inputs["W_h"], f32)
    W_qk = np.asarray(inputs["W_qk"], f32)
    W_o = np.asarray(inputs["W_o"], f32)
    whp = np.asarray(inputs["ln_h_g"], f32)[:, None] * W_h
    bhp = np.asarray(inputs["ln_h_b"], f32) @ W_h + np.asarray(inputs["b_h"], f32)
    wqkp = np.asarray(inputs["ln_qk_g"], f32)[:, None] * W_qk
    bqkp = np.asarray(inputs["ln_qk_b"], f32) @ W_qk + np.asarray(inputs["b_qk"], f32)
    wop = np.asarray(inputs["ln_o_g"], f32)[:, None] * W_o
    bop = np.asarray(inputs["ln_o_b"], f32) @ W_o + np.asarray(inputs["b_o"], f32)
    gamma = np.asarray(inputs["gamma"], f32).copy()
    beta = np.asarray(inputs["beta"], f32).copy()
    gamma[0] /= G
    beta[0] /= G
    gamma[3] /= N
    beta[3] /= N

    def lhsT(w, ktiles):
        return np.ascontiguousarray(w.reshape(ktiles, 128, -1).transpose(1, 0, 2)).astype(bf)

    def chan(v, ntiles):
        return np.ascontiguousarray(v.reshape(ntiles, 128).T).astype(f32)

    def dwl(dw, ntiles):
        return np.ascontiguousarray(
            dw.T.reshape(ntiles, 128, KTAPS).transpose(1, 0, 2)).astype(f32)

    return {
        "wh": lhsT(whp, 4), "wqk": lhsT(wqkp, 4), "wo": lhsT(wop, 8),
        "bh": chan(bhp, 16), "bqk": chan(bqkp, 1), "bo": chan(bop, 4),
        "dwh": dwl(np.asarray(inputs["dw_h"], f32), 16),
        "dwqk": dwl(np.asarray(inputs["dw_qk"], f32), 1),
        "dwo": dwl(np.asarray(inputs["dw_o"], f32), 4),
        "gb": np.concatenate([gamma.T, beta.T], axis=1).astype(f32),
    }


_NC = None


def get_nc():
    global _NC
    if _NC is None:
        _NC = _build_nc()
    return _NC


def make_in_maps(inputs):
    x = np.asarray(inputs["x"], np.float32)
    B = x.shape[0]
    prep = prep_inputs(inputs)
    return [{"x": np.ascontiguousarray(x[b]), **prep} for b in range(B)]


def kernel(**inputs):
    nc = get_nc()
    in_maps = make_in_maps(inputs)
    res = bass_utils.run_bass_kernel_spmd(nc, in_maps, core_ids=list(range(8)))
    out = np.stack([res.results[b]["out"] for b in range(8)], axis=0)
    return out.astype(np.float32)


# revision 39
# speedup vs baseline: 1.1395x; 1.0145x over previous
"""Self-contained TRN2 kernel for nn_FLASH_ShareA_FFConvM_FlashAttn.

kernel(**inputs) takes the full (unsharded) inputs from setup_inputs() and
returns the full (B, N, D) float32 output. Internally: data-parallel over the
batch — one batch sample per NeuronCore, 8 cores, no collectives.

v2: depthwise convs via fp8 DoubleRow diag matmuls on the PE; z_o kept
SBUF-resident (no DRAM roundtrip); batched Sqrt in the gating phase to avoid
activation-table thrash; shift copies on the scalar engine.
"""
import sys

if "/opt/trn_rl_repo" not in sys.path:
    sys.path.insert(0, "/opt/trn_rl_repo")

import numpy as np
import ml_dtypes
import concourse.bass as bass
import concourse.bacc as bacc
import concourse.mybir as mybir
import concourse.tile as tile
from concourse import bass_utils

F32 = mybir.dt.float32
BF16 = mybir.dt.bfloat16
FP8 = mybir.dt.float8e4
AF = mybir.ActivationFunctionType
OP = mybir.AluOpType
PM = mybir.MatmulPerfMode

N, D, H, QK, G = 4096, 512, 2048, 128, 256
NG = N // G
NT = N // 128
KTAPS = 17
PAD = 8
E2 = 2 * D
EPS = 1e-5
W8 = 2 * PAD + N + 16  # fp8 double-plane buffer width (4128, mult of 16)

# channel tiles of the depthwise convs on the PE (fp8 DoubleRow diag matmuls);
# the rest run on the vector engine. DVE tiles are scheduled early so no
# phase ends on a long vector-engine tail.
CONV_PE_HID = frozenset(range(16)) - {2, 5, 9}
CONV_PE_QK = True
CONV_PE_O = frozenset({0, 2, 3})
# DoubleRow tap pairs (k, k+2) matching a +2-column plane-1 shift
PAIR_KS = [0, 1, 4, 5, 8, 9, 12, 13]


def _conv_dve(nc, scratch, hpad, hpad1, dw_sb, dwi, acc):
    """acc = h + conv(h) via tensor_scalar products (4x) + tensor_tensor adds (2x)."""
    for k in range(KTAPS):
        s = k - PAD
        if s % 2 == 0:
            src, off = hpad, PAD + s
        else:
            src, off = hpad1, PAD - 1 + s
        if k == 0:
            nc.vector.scalar_tensor_tensor(
                out=acc[:, :], in0=src[:, off:off + N], scalar=dw_sb[:, dwi, 0:1],
                in1=hpad[:, PAD:PAD + N], op0=OP.mult, op1=OP.add)
        else:
            nc.vector.tensor_scalar(out=scratch[:, :], in0=src[:, off:off + N],
                                    scalar1=dw_sb[:, dwi, k:k + 1], scalar2=None,
                                    op0=OP.mult)
            nc.vector.tensor_add(acc[:, :], acc[:, :], scratch[:, :])


def _build_hp8(nc, pool, hpad):
    """fp8 double-plane buffer: plane0 = fp8(hpad), plane1 = plane0 shifted +2."""
    hp8 = pool.tile([128, 2, W8], FP8, tag="hp8")
    nc.scalar.activation(hp8[:, 0, 0:2 * PAD + N], hpad[:, :], AF.Copy)
    nc.vector.memset(hp8[:, 0, 2 * PAD + N:W8], 0.0)
    nc.sync.dma_start(hp8[:, 1, 0:W8 - 2], hp8[:, 0, 2:W8])
    nc.vector.memset(hp8[:, 1, W8 - 2:W8], 0.0)
    return hp8


def _build_dg(nc, pool, diag_mask, dw_sb, dwi):
    """fp8 diag weight pairs [128, 9, 2, 128] for DoubleRow conv."""
    dg = pool.tile([128, 9, 2, 128], FP8, tag="dg")
    for j, k in enumerate(PAIR_KS):
        nc.vector.tensor_scalar(out=dg[:, j, 0, :], in0=diag_mask[:, :],
                                scalar1=dw_sb[:, dwi, k:k + 1], scalar2=None, op0=OP.mult)
        nc.vector.tensor_scalar(out=dg[:, j, 1, :], in0=diag_mask[:, :],
                                scalar1=dw_sb[:, dwi, k + 2:k + 3], scalar2=None, op0=OP.mult)
    nc.vector.tensor_scalar(out=dg[:, 8, 0, :], in0=diag_mask[:, :],
                            scalar1=dw_sb[:, dwi, 16:17], scalar2=None, op0=OP.mult)
    nc.vector.memset(dg[:, 8, 1, :], 0.0)
    return dg


def _conv_dr(nc, psum_pool, hpad, hp8, dg, diag_mask, acc, evac):
    """acc = h + conv(h): bf16 identity + 9 fp8 DoubleRow diag pair-matmuls."""
    for half in range(4):
        ps0 = psum_pool.tile([128, 512], F32, tag="convdr")
        ps1 = psum_pool.tile([128, 512], F32, tag="convdr")
        ps = [ps0, ps1]
        bases = [PAD + (2 * half + i) * 512 for i in range(2)]
        for i in range(2):
            nc.tensor.matmul(ps[i][:, :], diag_mask[:, :], hpad[:, bases[i]:bases[i] + 512],
                             start=True, stop=False, skip_group_check=True)
        for j, k in enumerate(PAIR_KS + [16]):
            s = k - PAD
            last = j == 8
            for i in range(2):
                nc.tensor.matmul(ps[i][:, :], dg[:, j, :, :],
                                 hp8[:, :, bases[i] + s:bases[i] + s + 512],
                                 start=False, stop=last, perf_mode=PM.DoubleRow,
                                 skip_group_check=True)
        for i in range(2):
            c = 2 * half + i
            evac.activation(acc[:, c * 512:(c + 1) * 512], ps[i][:, :], AF.Copy)


def _emit(nc, tc, x, wh, wqk, wo, bh, bqk, bo, dwh, dwqk, dwo, gb, out, spill):
    consts = tc.alloc_tile_pool(name="consts", bufs=1)
    wqk_sb = consts.tile([128, 4, QK], BF16)
    nc.sync.dma_start(wqk_sb[:, :, :], wqk.ap())
    wo_sb = consts.tile([128, 8, D], BF16)
    nc.sync.dma_start(wo_sb[:, :, :], wo.ap())
    bh_sb = consts.tile([128, 16], F32)
    nc.sync.dma_start(bh_sb[:, :], bh.ap())
    bqk_sb = consts.tile([128, 1], F32)
    nc.sync.dma_start(bqk_sb[:, :], bqk.ap())
    bo_sb = consts.tile([128, 4], F32)
    nc.sync.dma_start(bo_sb[:, :], bo.ap())
    dwh_sb = consts.tile([128, 16, KTAPS], F32)
    nc.sync.dma_start(dwh_sb[:, :, :], dwh.ap())
    dwqk_sb = consts.tile([128, 1, KTAPS], F32)
    nc.sync.dma_start(dwqk_sb[:, :, :], dwqk.ap())
    dwo_sb = consts.tile([128, 4, KTAPS], F32)
    nc.sync.dma_start(dwo_sb[:, :, :], dwo.ap())
    gb_sb = consts.tile([128, 8], F32)
    nc.sync.dma_start(gb_sb[:, :], gb.ap())
    eps_sb = consts.tile([128, 1], F32)
    nc.vector.memset(eps_sb[:, :], EPS)

    iota_row = consts.tile([128, 128], F32)
    nc.gpsimd.iota(iota_row[:, :], pattern=[[1, 128]], base=0, channel_multiplier=0,
                   allow_small_or_imprecise_dtypes=True)
    iota_p = consts.tile([128, 1], F32)
    nc.gpsimd.iota(iota_p[:, :], pattern=[[0, 1]], base=0, channel_multiplier=1,
                   allow_small_or_imprecise_dtypes=True)
    diag_mask = consts.tile([128, 128], BF16)
    nc.vector.tensor_scalar(out=diag_mask[:, :], in0=iota_row[:, :],
                            scalar1=iota_p[:, :], scalar2=None, op0=OP.is_equal)

    p03 = tc.alloc_tile_pool(name="p03", bufs=1)     # P0-P3: zT
    zT = p03.tile([128, 4, N], BF16)
    qs = tc.alloc_tile_pool(name="qside", bufs=1, side="right")    # P1-P4
    attnT = qs.tile([128, NG, 2, G], BF16)
    lq_sb = qs.tile([128, N], BF16)
    lk_str = qs.tile([128, NT, 128], BF16)
    linkv_sb = qs.tile([128, E2], BF16)
    linku_sb = qs.tile([128, E2], BF16)

    # P0 + P1 head: token-shifted LayerNorm, with the qk GEMM interleaved
    # per 512-token chunk (keeps the PE warm during the load/normalize phase).
    with tc.tile_pool(name="p0", bufs=3) as p0, \
         tc.tile_pool(name="p0s", bufs=8) as p0s, \
         tc.tile_pool(name="p1", bufs=1) as p1, \
         tc.tile_pool(name="p1p", bufs=2, space="PSUM") as p1p:
        qkp = p1.tile([128, 2 * PAD + N], BF16, tag="qkpad")
        nc.vector.memset(qkp[:, 0:PAD], 0.0)
        nc.vector.memset(qkp[:, PAD + N:], 0.0)
        for ch in range(8):
            t0 = ch * 512
            xt4 = p0.tile([128, 4, D], F32, tag="xt4")
            if ch == 0:
                nc.vector.memset(xt4[0:1, 0, 0:D // 2], 0.0)
                nc.sync.dma_start(xt4[1:128, 0, 0:D // 2], x[0:127, 0:D // 2])
                nc.sync.dma_start(
                    xt4[:, 1:4, 0:D // 2],
                    x[127:511, 0:D // 2].rearrange("(q p) d -> p q d", p=128))
            else:
                nc.sync.dma_start(
                    xt4[:, :, 0:D // 2],
                    x[t0 - 1:t0 + 511, 0:D // 2].rearrange("(q p) d -> p q d", p=128))
            nc.sync.dma_start(
                xt4[:, :, D // 2:D],
                x[t0:t0 + 512, D // 2:D].rearrange("(q p) d -> p q d", p=128))
            mv4 = p0s.tile([128, 4, 2], F32, tag="mv4p0")
            for q in range(4):
                st6 = p0s.tile([128, 6], F32, tag="st6")
                nc.vector.bn_stats(st6[:, :], xt4[:, q, :])
                nc.vector.bn_aggr(mv4[:, q, :], st6[:, :])
            rstd4 = p0s.tile([128, 4], F32, tag="rstd4p0")
            nc.scalar.activation(rstd4[:, :], mv4[:, :, 1], AF.Sqrt, bias=eps_sb[:, :],
                                 scale=1.0)
            nc.vector.reciprocal(rstd4[:, :], rstd4[:, :])
            for q in range(4):
                nmu = p0s.tile([128, 1], F32, tag="nmu")
                nc.vector.tensor_scalar(out=nmu[:, :], in0=mv4[:, q, 0:1],
                                        scalar1=rstd4[:, q:q + 1],
                                        scalar2=-1.0, op0=OP.mult, op1=OP.mult)
                zt = p0.tile([128, D], BF16, tag="zt")
                nc.vector.tensor_scalar(out=zt[:, :], in0=xt4[:, q, :],
                                        scalar1=rstd4[:, q:q + 1],
                                        scalar2=nmu[:, :], op0=OP.mult, op1=OP.add)
                nc.sync.dma_start_transpose(zT[:, :, t0 + q * 128:t0 + (q + 1) * 128],
                                            zt[:, :])
            ps = p1p.tile([128, 512], F32, tag="qkps")
            for kt in range(4):
                nc.tensor.matmul(ps[:, :], wqk_sb[:, kt, :], zT[:, kt, ch * 512:(ch + 1) * 512],
                                 start=(kt == 0), stop=(kt == 3))
            nc.scalar.activation(qkp[:, PAD + ch * 512:PAD + (ch + 1) * 512], ps[:, :],
                                 AF.Silu, bias=bqk_sb[:, :], scale=1.0)

        qkc = p1.tile([128, N], BF16, tag="qkc")
        if CONV_PE_QK:
            with tc.tile_pool(name="p1cp", bufs=2, space="PSUM") as p1cp:
                hp8 = _build_hp8(nc, p1, qkp)
                dg = _build_dg(nc, p1, diag_mask, dwqk_sb, 0)
                _conv_dr(nc, p1cp, qkp, hp8, dg, diag_mask, qkc, nc.scalar)
        else:
            qkp1 = p1.tile([128, 2 * PAD + N], BF16, tag="qkpad1")
            nc.scalar.activation(qkp1[:, 0:2 * PAD + N - 2], qkp[:, 1:2 * PAD + N - 1], AF.Copy)
            qscr = p1.tile([128, N], BF16, tag="qscr")
            _conv_dve(nc, qscr, qkp, qkp1, dwqk_sb, 0, qkc)
        qq = p1.tile([128, N], BF16, tag="qq")
        qkk = p1.tile([128, N], BF16, tag="qkk")
        lkk = p1.tile([128, N], BF16, tag="lkk")
        for i, dst in ((0, qq), (1, lq_sb), (2, qkk), (3, lkk)):
            nc.vector.tensor_scalar(out=dst[:, :], in0=qkc[:, :], scalar1=gb_sb[:, i:i + 1],
                                    scalar2=gb_sb[:, 4 + i:5 + i], op0=OP.mult, op1=OP.add)
        nc.sync.dma_start_transpose(lk_str[:, :, :], lkk[:, :])

        for g in range(NG):
            for jh in range(2):
                sp = p1p.tile([128, G], F32, tag="simps")
                nc.tensor.matmul(sp[:, :], qkk[:, g * G + jh * 128: g * G + jh * 128 + 128],
                                 qq[:, g * G:(g + 1) * G], start=True, stop=True)
                rel = p1.tile([128, G], BF16, tag="rel")
                nc.scalar.activation(rel[:, :], sp[:, :], AF.Relu)
                nc.vector.tensor_mul(attnT[:, g, jh, :], rel[:, :], rel[:, :])

    # P3: hidden + conv + spill + lin_kv/lin_ku
    spill_v = spill.ap().rearrange("(tt p) (q c4) -> p tt q c4", p=128, c4=512)
    with tc.tile_pool(name="p3w", bufs=4) as p3w, \
         tc.tile_pool(name="p3h", bufs=3) as p3h, \
         tc.tile_pool(name="p3", bufs=2) as p3, \
         tc.tile_pool(name="p3q", bufs=1) as p3q, \
         tc.tile_pool(name="p3p", bufs=2, space="PSUM") as p3p, \
         tc.tile_pool(name="p3c", bufs=2, space="PSUM") as p3c, \
         tc.tile_pool(name="p3lin", bufs=2, space="PSUM") as p3lin:
        state = {"strips4": None}

        def produce(hc):
            wt = p3w.tile([128, 4, 128], BF16, tag="wt")
            nc.sync.dma_start(wt[:, :, :], wh[:, :, hc * 128:(hc + 1) * 128])
            hpad = p3h.tile([128, 2 * PAD + N], BF16, tag="hpad")
            nc.vector.memset(hpad[:, 0:PAD], 0.0)
            nc.vector.memset(hpad[:, PAD + N:], 0.0)
            for cp2 in range(4):
                c0 = 2 * cp2
                ps = p3p.tile([128, 1024], F32, tag="hps")
                for kt in range(4):
                    nc.tensor.matmul(ps[:, 0:512], wt[:, kt, :],
                                     zT[:, kt, c0 * 512:(c0 + 1) * 512],
                                     start=(kt == 0), stop=(kt == 3))
                    nc.tensor.matmul(ps[:, 512:1024], wt[:, kt, :],
                                     zT[:, kt, (c0 + 1) * 512:(c0 + 2) * 512],
                                     start=(kt == 0), stop=(kt == 3))
                nc.scalar.activation(hpad[:, PAD + c0 * 512:PAD + (c0 + 2) * 512], ps[:, :],
                                     AF.Silu, bias=bh_sb[:, hc:hc + 1], scale=1.0)
            return hpad

        def convpost(hc, hpad):
            if hc % 4 == 0:
                s4_new = p3q.tile([128, NT, 4, 128], BF16, tag="strips4")
                state["strips4"] = s4_new
            strips4 = state["strips4"]
            acc = p3.tile([128, N], BF16, tag="acc")
            if hc in CONV_PE_HID:
                hp8 = _build_hp8(nc, p3, hpad)
                dg = _build_dg(nc, p3, diag_mask, dwh_sb, hc)
                _conv_dr(nc, p3c, hpad, hp8, dg, diag_mask, acc, nc.scalar)
            else:
                hpad1 = p3q.tile([128, 2 * PAD + N], BF16, tag="hpad1")
                nc.scalar.activation(hpad1[:, 0:2 * PAD + N - 2], hpad[:, 1:2 * PAD + N - 1],
                                     AF.Copy)
                scr = p3q.tile([128, N], BF16, tag="convscr")
                _conv_dve(nc, scr, hpad, hpad1, dwh_sb, hc, acc)
            nc.sync.dma_start_transpose(strips4[:, :, hc % 4, :], acc[:, :])
            if hc % 4 == 3:
                q = hc // 4
                nc.sync.dma_start(spill_v[:, :, q, :], strips4[:, :, :, :])
                dst = linkv_sb if hc < 8 else linku_sb
                col = (q % 2) * 512
                lps = p3lin.tile([128, 512], F32, tag="linps")
                for tt in range(NT):
                    nc.tensor.matmul(
                        lps[:, :], lk_str[:, tt, :],
                        strips4[:, tt, :, :].rearrange("p a c -> p (a c)"),
                        start=(tt == 0), stop=(tt == NT - 1))
                nc.scalar.activation(dst[:, col:col + 512], lps[:, :], AF.Copy)

        pending = []
        for hc in range(16):
            pending.append((hc, produce(hc)))
            if len(pending) > 2:
                convpost(*pending.pop(0))
        for item in pending:
            convpost(*item)
    p03.release()

    # P4: attention + gating + LN_o ; z_o kept SBUF-resident (transposed)
    pz = tc.alloc_tile_pool(name="pz", bufs=1)
    z_oT = pz.tile([128, 8, N], BF16)
    with tc.tile_pool(name="p4", bufs=2) as p4, \
         tc.tile_pool(name="p4g", bufs=2) as p4g, \
         tc.tile_pool(name="p4s", bufs=8) as p4s, \
         tc.tile_pool(name="p4p", bufs=2, space="PSUM") as p4p:
        for gpair in range(NG // 2):
            # batch = 2 groups x 2 it-tiles = 4 token tiles; deferred sqrt
            gos = []
            mv4 = p4s.tile([128, 4, 2], F32, tag="mv4")
            vgs, ugs = {}, {}
            for half_g in range(2):
                g = gpair * 2 + half_g
                vg, ug = [], []
                for jh in range(2):
                    vt = p4.tile([128, E2], BF16, tag=f"vg{half_g}{jh}")
                    nc.sync.dma_start(vt[:, :], spill[g * G + jh * 128: g * G + jh * 128 + 128, 0:E2])
                    ut = p4.tile([128, E2], BF16, tag=f"ug{half_g}{jh}")
                    nc.sync.dma_start(ut[:, :], spill[g * G + jh * 128: g * G + jh * 128 + 128, E2:H])
                    vg.append(vt)
                    ug.append(ut)
                vgs[g], ugs[g] = vg, ug
            for j in range(4):
                g = gpair * 2 + j // 2
                it = j % 2
                vg, ug = vgs[g], ugs[g]
                ap_ = p4p.tile([128, 2 * E2], F32, tag="attps")
                islice = slice(g * G + it * 128, g * G + it * 128 + 128)
                for half, (grp, lin) in enumerate(((vg, linkv_sb), (ug, linku_sb))):
                    base = half * E2
                    for e in range(2):
                        for jh in range(2):
                            nc.tensor.matmul(ap_[:, base + e * 512:base + (e + 1) * 512],
                                             attnT[:, g, jh, it * 128:it * 128 + 128],
                                             grp[jh][:, e * 512:(e + 1) * 512],
                                             start=(jh == 0), stop=False)
                        nc.tensor.matmul(ap_[:, base + e * 512:base + (e + 1) * 512],
                                         lq_sb[:, islice], lin[:, e * 512:(e + 1) * 512],
                                         start=False, stop=True)
                avau = p4.tile([128, 2 * E2], BF16, tag="avau")
                nc.scalar.activation(avau[:, :], ap_[:, :], AF.Copy)
                t1 = p4.tile([128, E2], BF16, tag="t1")
                nc.vector.tensor_mul(t1[:, :], ug[it][:, :], avau[:, 0:E2])
                sg = p4.tile([128, E2], BF16, tag="sg")
                nc.scalar.activation(sg[:, :], t1[:, :], AF.Sigmoid)
                t2 = p4.tile([128, E2], BF16, tag="t2")
                nc.gpsimd.tensor_mul(t2[:, :], vg[it][:, :], avau[:, E2:2 * E2])
                go = p4g.tile([128, E2], BF16, tag=f"go{j}")
                sumg = p4s.tile([128, 1], F32, tag="sumg")
                nc.vector.scalar_tensor_tensor(out=go[:, :], in0=t2[:, :], scalar=1.0,
                                               in1=sg[:, :], op0=OP.mult, op1=OP.mult,
                                               accum_out=sumg[:, :])
                g2 = p4.tile([128, E2], BF16, tag="g2")
                sumg2 = p4s.tile([128, 1], F32, tag="sumg2")
                nc.scalar.activation(g2[:, :], go[:, :], AF.Square, accum_out=sumg2[:, :])
                nc.vector.tensor_scalar_mul(mv4[:, j, 0:1], sumg[:, :], 1.0 / E2)
                mm = p4s.tile([128, 1], F32, tag="mm")
                nc.vector.tensor_scalar(out=mm[:, :], in0=mv4[:, j, 0:1], scalar1=mv4[:, j, 0:1],
                                        scalar2=-1.0, op0=OP.mult, op1=OP.mult)
                nc.vector.tensor_scalar(out=mv4[:, j, 1:2], in0=sumg2[:, :], scalar1=1.0 / E2,
                                        scalar2=mm[:, :], op0=OP.mult, op1=OP.add)
                gos.append(go)
            # batched rsqrt over the 4 tiles (one Sqrt act -> one table load)
            rstd4 = p4s.tile([128, 4], F32, tag="rstd4")
            nc.scalar.activation(rstd4[:, :], mv4[:, :, 1], AF.Sqrt, bias=eps_sb[:, :], scale=1.0)
            nc.vector.reciprocal(rstd4[:, :], rstd4[:, :])
            for j in range(4):
                g = gpair * 2 + j // 2
                it = j % 2
                nmu = p4s.tile([128, 1], F32, tag="nmu4")
                nc.vector.tensor_scalar(out=nmu[:, :], in0=mv4[:, j, 0:1],
                                        scalar1=rstd4[:, j:j + 1],
                                        scalar2=-1.0, op0=OP.mult, op1=OP.mult)
                zo = p4.tile([128, E2], BF16, tag="zo")
                nc.vector.tensor_scalar(out=zo[:, :], in0=gos[j][:, :], scalar1=rstd4[:, j:j + 1],
                                        scalar2=nmu[:, :], op0=OP.mult, op1=OP.add)
                tti = g * 2 + it
                nc.sync.dma_start_transpose(z_oT[:, :, tti * 128:(tti + 1) * 128], zo[:, :])
    qs.release()

    # P5: output FFConvM reading z_oT from SBUF
    pvo = tc.alloc_tile_pool(name="pvo", bufs=1, side="right")
    vo_big = pvo.tile([128, NT, 4, 128], BF16)
    with tc.tile_pool(name="p5", bufs=2) as p5, \
         tc.tile_pool(name="p5q", bufs=1) as p5q, \
         tc.tile_pool(name="p5p", bufs=2, space="PSUM") as p5p, \
         tc.tile_pool(name="p5c", bufs=2, space="PSUM") as p5c:

        def produce5(oc):
            hpad = p5.tile([128, 2 * PAD + N], BF16, tag="hpad5")
            nc.vector.memset(hpad[:, 0:PAD], 0.0)
            nc.vector.memset(hpad[:, PAD + N:], 0.0)
            for cp2 in range(4):
                c0 = 2 * cp2
                ps = p5p.tile([128, 1024], F32, tag="ops")
                for kt in range(8):
                    nc.tensor.matmul(ps[:, 0:512], wo_sb[:, kt, oc * 128:(oc + 1) * 128],
                                     z_oT[:, kt, c0 * 512:(c0 + 1) * 512],
                                     start=(kt == 0), stop=(kt == 7))
                    nc.tensor.matmul(ps[:, 512:1024], wo_sb[:, kt, oc * 128:(oc + 1) * 128],
                                     z_oT[:, kt, (c0 + 1) * 512:(c0 + 2) * 512],
                                     start=(kt == 0), stop=(kt == 7))
                nc.scalar.activation(hpad[:, PAD + c0 * 512:PAD + (c0 + 2) * 512], ps[:, :],
                                     AF.Silu, bias=bo_sb[:, oc:oc + 1], scale=1.0)
            return hpad

        def convpost5(oc, hpad):
            acc = p5.tile([128, N], BF16, tag="acc5")
            if oc in CONV_PE_O:
                hp8 = _build_hp8(nc, p5, hpad)
                dg = _build_dg(nc, p5, diag_mask, dwo_sb, oc)
                _conv_dr(nc, p5c, hpad, hp8, dg, diag_mask, acc, nc.scalar)
            else:
                hpad1 = p5q.tile([128, 2 * PAD + N], BF16, tag="hpad51")
                nc.scalar.activation(hpad1[:, 0:2 * PAD + N - 2], hpad[:, 1:2 * PAD + N - 1],
                                     AF.Copy)
                scr = p5q.tile([128, N], BF16, tag="convscr5")
                _conv_dve(nc, scr, hpad, hpad1, dwo_sb, oc, acc)
            nc.sync.dma_start_transpose(vo_big[:, :, oc, :], acc[:, :])

        prev = None
        for oc in range(4):
            hp = produce5(oc)
            if prev is not None:
                convpost5(*prev)
            prev = (oc, hp)
        convpost5(*prev)
    pz.release()

    # P6: residual, 4 token-tiles per DMA; x prefetched during P5, adds split
    # across the vector and gpsimd engines to shorten the tail.
    with tc.tile_pool(name="p6x", bufs=1) as p6x, \
         tc.tile_pool(name="p6", bufs=4) as p6:
        xts = []
        for c in range(NT // 4):
            t0 = c * 512
            xt = p6x.tile([128, 4, D], F32, tag=f"xt6{c}")
            nc.sync.dma_start(xt[:, :, :],
                              x[t0:t0 + 512, :].rearrange("(q p) d -> p q d", p=128))
            xts.append(xt)
        for c in range(NT // 4):
            t0 = c * 512
            of = p6.tile([128, 4, D], F32, tag="of")
            eng = nc.vector if c % 2 == 0 else nc.gpsimd
            eng.tensor_add(
                of[:, :, :].rearrange("p q d -> p (q d)"),
                xts[c][:, :, :].rearrange("p q d -> p (q d)"),
                vo_big[:, 4 * c:4 * c + 4, :, :].rearrange("p q a c -> p (q a c)"))
            nc.sync.dma_start(out[t0:t0 + 512, :].rearrange("(q p) d -> p q d", p=128),
                              of[:, :, :])
    pvo.release()
    consts.release()


def _build_nc():
    nc = bacc.Bacc("TRN2", target_bir_lowering=False, debug=False)
    x = nc.dram_tensor("x", [N, D], F32, kind="ExternalInput")
    wh = nc.dram_tensor("wh", [128, 4, H], BF16, kind="ExternalInput")
    wqk = nc.dram_tensor("wqk", [128, 4, QK], BF16, kind="ExternalInput")
    wo = nc.dram_tensor("wo", [128, 8, D], BF16, kind="ExternalInput")
    bh = nc.dram_tensor("bh", [128, 16], F32, kind="ExternalInput")
    bqk = nc.dram_tensor("bqk", [128, 1], F32, kind="ExternalInput")
    bo = nc.dram_tensor("bo", [128, 4], F32, kind="ExternalInput")
    dwh = nc.dram_tensor("dwh", [128, 16, KTAPS], F32, kind="ExternalInput")
    dwqk = nc.dram_tensor("dwqk", [128, 1, KTAPS], F32, kind="ExternalInput")
    dwo = nc.dram_tensor("dwo", [128, 4, KTAPS], F32, kind="ExternalInput")
    gb = nc.dram_tensor("gb", [128, 8], F32, kind="ExternalInput")
    out = nc.dram_tensor("out", [N, D], F32, kind="ExternalOutput")
    spill = nc.dram_tensor("spill", [N, H], BF16)
    with tile.TileContext(nc) as tc:
        _emit(nc, tc, x, wh, wqk, wo, bh, bqk, bo, dwh, dwqk, dwo, gb, out, spill)
    nc.compile()
    return nc


def prep_inputs(inputs):
    f32 = np.float32
    bf = ml_dtypes.bfloat16
    W_h = np.asarray(inputs["W_h"], f32)
    W_qk = np.asarray(inputs["W_qk"], f32)
    W_o = np.asarray(inputs["W_o"], f32)
    whp = np.asarray(inputs["ln_h_g"], f32)[:, None] * W_h
    bhp = np.asarray(inputs["ln_h_b"], f32) @ W_h + np.asarray(inputs["b_h"], f32)
    wqkp = np.asarray(inputs["ln_qk_g"], f32)[:, None] * W_qk
    bqkp = np.asarray(inputs["ln_qk_b"], f32) @ W_qk + np.asarray(inputs["b_qk"], f32)
    wop = np.asarray(inputs["ln_o_g"], f32)[:, None] * W_o
    bop = np.asarray(inputs["ln_o_b"], f32) @ W_o + np.asarray(inputs["b_o"], f32)
    gamma = np.asarray(inputs["gamma"], f32).copy()
    beta = np.asarray(inputs["beta"], f32).copy()
    gamma[0] /= G
    beta[0] /= G
    gamma[3] /= N
    beta[3] /= N

    def lhsT(w, ktiles):
        return np.ascontiguousarray(w.reshape(ktiles, 128, -1).transpose(1, 0, 2)).astype(bf)

    def chan(v, ntiles):
        return np.ascontiguousarray(v.reshape(ntiles, 128).T).astype(f32)

    def dwl(dw, ntiles):
        return np.ascontiguousarray(
            dw.T.reshape(ntiles, 128, KTAPS).transpose(1, 0, 2)).astype(f32)

    return {
        "wh": lhsT(whp, 4), "wqk": lhsT(wqkp, 4), "wo": lhsT(wop, 8),
        "bh": chan(bhp, 16), "bqk": chan(bqkp, 1), "bo": chan(bop, 4),
        "dwh": dwl(np.asarray(inputs["dw_h"], f32), 16),
        "dwqk": dwl(np.asarray(inputs["dw_qk"], f32), 1),
        "dwo": dwl(np.asarray(inputs["dw_o"], f32), 4),
        "gb": np.concatenate([gamma.T, beta.T], axis=1).astype(f32),
    }


_NC = None


def get_nc():
    global _NC
    if _NC is None:
        _NC = _build_nc()
    return _NC


def make_in_maps(inputs):
    x = np.asarray(inputs["x"], np.float32)
    B = x.shape[0]
    prep = prep_inputs(inputs)
    return [{"x": np.ascontiguousarray(x[b]), **prep} for b in range(B)]


def kernel(**inputs):
    nc = get_nc()
    in_maps = make_in_maps(inputs)
    res = bass_utils.run_bass_kernel_spmd(nc, in_maps, core_ids=list(range(8)))
    out = np.stack([res.results[b]["out"] for b in range(8)], axis=0)
    return out.astype(np.float32)
